# revision 1
# baseline (speedup 1.0000x reference)
"""DCRNN (PEMS-BAY) Trainium2 Bass kernel, data-parallel over batch on 8 cores.

Layouts per core (local batch BL=8):
  A-layout: [feature partitions, b*384 + n]  (n padded 325->384; 8*384 = 3072 cols)
  B-layout: [node-chunk partitions (128/128/69), b*Fout + f]
gconv (W-first):  out = X@A0 + S @ (X@W1 + S @ (X@(2*W2)))
  P2,P1 = W-matmuls in A-layout; transpose down to B; X1 = S@P2B; Q = X1+P1B;
  R = S@QB; PE-transposes of R accumulate onto the X@A0 PSUM banks; activation
  evacuates PSUM -> SBUF.
State tile XH per layer: rows 0:64 = h, rows 64:128 = x (padded features).
"""
import sys
import os
import numpy as np

sys.path.insert(0, "/opt/trn_rl_repo")

import concourse.bass as bass  # noqa: E402
import concourse.mybir as mybir  # noqa: E402
import concourse.tile as tile  # noqa: E402
from concourse import bacc  # noqa: E402
from concourse.bass_utils import run_bass_kernel_spmd  # noqa: E402
from concourse.masks import make_identity  # noqa: E402

# problem constants
N = 325
B = 64
T = 12
HZ = 12
U = 64
DIN = 2
DOUT = 1
NCORES = 8
BL = B // NCORES          # 8 local batch
NB = 384                  # padded node stride per batch
AF = BL * NB              # 3072 A-layout free width
NCH = [(0, 128), (128, 128), (256, 69)]   # node chunks (offset, len)
NBANK = AF // 512         # 6 psum banks for a full A row

F32 = mybir.dt.float32
MMDT = mybir.dt.float32r  # matmul input dtype (float32 | float32r)
AFT = mybir.ActivationFunctionType

CELLS = ["enc0", "enc1", "dec0", "dec1"]
CELL_DIN = {"enc0": DIN, "enc1": U, "dec0": DOUT, "dec1": U}

_BUILD_CACHE = {}
LAST_RESULT = None


def _install_ntff_hook():
    """Register the axon NTFF profiling hook if the image lacks antenv.axon_hooks."""
    import types
    import antenv
    if getattr(antenv, "axon_hooks", None) is not None:
        return
    m = types.ModuleType("antenv.axon_hooks")
    state = {"h": None}
    m.set_axon_ntff_profile_hook = lambda h: state.__setitem__("h", h)
    m.get_axon_ntff_profile_hook = lambda: state["h"]
    sys.modules["antenv.axon_hooks"] = m
    antenv.axon_hooks = m
    try:
        from trn_agent_boot.trn_boot import _ntff_profile_via_ctypes
        hook = _ntff_profile_via_ctypes("/opt/axon/libaxon_pjrt.so")
        if hook is not None:
            m.set_axon_ntff_profile_hook(hook)
    except Exception:
        pass


def _pad_w(w, din, fout):
    """(3F, fout) -> three [128, fout] padded mats A0, W1, 2*W2.

    Padded row map: rows 0:64 <- h/rh features (orig rows din:F),
    rows 64:64+din <- x features (orig rows 0:din). Others zero.
    """
    f = din + U
    w0, w1, w2 = w[0:f], w[f:2 * f], w[2 * f:3 * f]

    def pad(m):
        p = np.zeros((128, fout), np.float32)
        p[0:64] = m[din:f]
        p[64:64 + din] = m[0:din]
        return p

    return pad(w0 - w2), pad(w1), pad(2.0 * w2)


def _build(nsteps_enc, nsteps_dec):
    key = (nsteps_enc, nsteps_dec)
    if key in _BUILD_CACHE:
        return _BUILD_CACHE[key]

    nc = bacc.Bacc()
    # ---- DRAM params ----
    x_in = nc.declare_dram_parameter("x", [T, DIN, AF], MMDT, isOutput=False)
    s_in = nc.declare_dram_parameter("s", [N, N], MMDT, isOutput=False)
    wparams = {}
    for c in CELLS:
        for nm, shp in [("gA0", [128, 128]), ("gW1", [128, 128]),
                        ("gW2", [128, 128]), ("cA0", [128, 64]),
                        ("cW1", [128, 64]), ("cW2", [128, 64]),
                        ("gb", [128, 1]), ("cb", [64, 1])]:
            dt_ = F32 if nm in ("gb", "cb") else MMDT
            wparams[f"{c}_{nm}"] = nc.declare_dram_parameter(
                f"{c}_{nm}", shp, dt_, isOutput=False)
    wparams["pW"] = nc.declare_dram_parameter("pW", [64, 1], MMDT, isOutput=False)
    wparams["pb"] = nc.declare_dram_parameter("pb", [1, 1], F32, isOutput=False)
    out_d = nc.declare_dram_parameter("out", [HZ, 1, BL, N], F32, isOutput=True)

    with tile.TileContext(nc) as tc:
        with tc.tile_pool(name="const", bufs=1) as cp, \
             tc.tile_pool(name="state", bufs=1) as st, \
             tc.tile_pool(name="pa", bufs=1) as pa, \
             tc.tile_pool(name="bp", bufs=1) as bp, \
             tc.tile_pool(name="pstagep", bufs=6, space="PSUM") as psp:

            # ---- constants to SBUF ----
            wt = {}
            for c in CELLS:
                for nm in ["gA0", "gW1", "gW2"]:
                    wt[f"{c}_{nm}"] = cp.tile([128, 128], MMDT, tag=f"{c}_{nm}", name=f"{c}_{nm}")
                for nm in ["cA0", "cW1", "cW2"]:
                    wt[f"{c}_{nm}"] = cp.tile([128, 64], MMDT, tag=f"{c}_{nm}", name=f"{c}_{nm}")
                wt[f"{c}_gb"] = cp.tile([128, 1], F32, tag=f"{c}_gb", name=f"{c}_gb")
                wt[f"{c}_cb"] = cp.tile([64, 1], F32, tag=f"{c}_cb", name=f"{c}_cb")
            wt["pW"] = cp.tile([64, 1], MMDT, tag="pW", name="pW")
            wt["pb"] = cp.tile([1, 1], F32, tag="pb", name="pb")
            for k, t in wt.items():
                nc.sync.dma_start(out=t, in_=wparams[k][:])
            s_t = []
            for ci, (c0, cl) in enumerate(NCH):
                stl = cp.tile([128, N], MMDT, tag=f"s{ci}", name=f"s{ci}")
                nc.sync.dma_start(out=stl[0:cl, :], in_=s_in[c0:c0 + cl, :])
                s_t.append(stl)
            ident = cp.tile([128, 128], F32, tag="ident")
            make_identity(nc, ident)

            # ---- state tiles (split per batch-half for pipeline overlap) ----
            HB = BL // 2          # 4 batches per half
            HAF = HB * NB         # 1536 A-cols per half
            xh = {}
            xrh, r_h, u_h, c_h = {}, {}, {}, {}
            for hf in range(2):
                for c in CELLS:
                    xh[(c, hf)] = st.tile([128, HAF], MMDT, tag=f"xh_{c}_{hf}",
                                          name=f"xh_{c}_{hf}")
                xrh[hf] = st.tile([128, HAF], MMDT, tag=f"xr{hf}", name=f"xr{hf}")
                r_h[hf] = st.tile([64, HAF], F32, tag=f"r{hf}", name=f"r{hf}")
                u_h[hf] = st.tile([64, HAF], F32, tag=f"u{hf}", name=f"u{hf}")
                c_h[hf] = st.tile([64, HAF], F32, tag=f"c{hf}", name=f"c{hf}")

            for tl in [xh[k] for k in xh] + [xrh[0], xrh[1]]:
                nc.vector.memset(tl[:, :].bitcast(F32), 0.0)
            tc.strict_bb_all_engine_barrier()

            def gconv(cell, rhs_t, wprefix, fout, hf):
                """One gconv on one batch-half. rhs_t: [128, HAF] MMDT tile.
                Returns preact psum tile [128, HAF] (rows 0:fout valid)."""
                a0w = wt[f"{cell}_{wprefix}A0"]
                w1 = wt[f"{cell}_{wprefix}W1"]
                w2 = wt[f"{cell}_{wprefix}W2"]
                fhh = HB * fout           # B free width per half (512/256)

                p2a = pa.tile([128, HAF], F32, tag=f"p2a{hf}", name=f"p2a{hf}")
                p1a = pa.tile([128, HAF], F32, tag=f"p1a{hf}", name=f"p1a{hf}")
                for w_, dst in ((w2, p2a), (w1, p1a)):
                    for ci in range(3):
                        sl = slice(ci * 512, (ci + 1) * 512)
                        pt = psp.tile([128, 512], F32, tag="pstage")
                        nc.tensor.matmul(pt[0:fout, :], w_[0:128, :],
                                         rhs_t[:, sl], start=True, stop=True)
                        nc.scalar.copy(dst[0:fout, sl], pt[0:fout, :])

                # P0 -> SBUF
                p0a = pa.tile([128, HAF], F32, tag=f"p0a{hf}", name=f"p0a{hf}")
                for ci in range(3):
                    sl = slice(ci * 512, (ci + 1) * 512)
                    pt = psp.tile([128, 512], F32, tag="pstage")
                    nc.tensor.matmul(pt[0:fout, :], a0w[0:128, :],
                                     rhs_t[:, sl], start=True, stop=True)
                    nc.scalar.copy(p0a[0:fout, sl], pt[0:fout, :])

                # down-transposes P2A,P1A -> B-layout
                bt = {}
                for role, srct in (("p2b", p2a), ("p1b", p1a)):
                    dt_ = MMDT if role == "p2b" else F32
                    tiles = [bp.tile([128, 512], dt_, tag=f"{role}{ci}_{hf}",
                                     name=f"{role}{ci}_{hf}") for ci in range(3)]
                    for ci, (c0, cl) in enumerate(NCH):
                        dt = psp.tile([128, 512], F32, tag="pstage")
                        for j in range(HB):
                            nc.tensor.matmul(
                                dt[0:cl, j * fout:(j + 1) * fout],
                                srct[0:fout, j * NB + c0: j * NB + c0 + cl],
                                ident[0:fout, 0:fout], is_transpose=True,
                                start=(j == 0), stop=(j == HB - 1))
                        if role == "p2b":
                            nc.vector.tensor_copy(tiles[ci][0:cl, 0:fhh],
                                                  dt[0:cl, 0:fhh])
                        else:
                            nc.scalar.copy(tiles[ci][0:cl, 0:fhh],
                                           dt[0:cl, 0:fhh])
                    bt[role] = tiles

                # X1 = S@P2B ; Q = X1 + P1B ; R = S@QB
                qb = [bp.tile([128, 512], MMDT, tag=f"p2b{ci}_{hf}",
                              name=f"qb{ci}_{hf}") for ci in range(3)]
                rbt = [bp.tile([128, 512], F32, tag=f"p1b{ci}_{hf}",
                               name=f"rb{ci}_{hf}") for ci in range(3)]
                for dst, srcs, srcadd in ((qb, bt["p2b"], bt["p1b"]),
                                          (rbt, qb, None)):
                    for mi, (m0, ml) in enumerate(NCH):
                        xt = psp.tile([128, 512], F32, tag="pstage")
                        for ki, (k0, kl) in enumerate(NCH):
                            nc.tensor.matmul(
                                xt[0:ml, 0:fhh], s_t[ki][0:kl, m0:m0 + ml],
                                srcs[ki][0:kl, 0:fhh],
                                start=(ki == 0), stop=(ki == 2))
                        if srcadd is not None:
                            nc.vector.tensor_tensor(
                                dst[mi][0:ml, 0:fhh], xt[0:ml, 0:fhh],
                                srcadd[mi][0:ml, 0:fhh], mybir.AluOpType.add)
                        else:
                            nc.scalar.copy(dst[mi][0:ml, 0:fhh],
                                           xt[0:ml, 0:fhh])

                # up-transposes RB -> psum bank; add P0A -> SBUF preact
                preact = pa.tile([128, HAF], F32, tag=f"p2a{hf}",
                                 name=f"pre{hf}")
                blocks_by_bank = {}
                for b in range(HB):
                    for ci in range(3):
                        c0, cl = NCH[ci]
                        blocks_by_bank.setdefault((b * NB + c0) // 512,
                                                  []).append((b, ci))
                for bk, blks in blocks_by_bank.items():
                    ut = psp.tile([128, 512], F32, tag="pstage")
                    for j, (b, ci) in enumerate(blks):
                        c0, cl = NCH[ci]
                        off = b * NB + c0 - bk * 512
                        nc.tensor.matmul(
                            ut[0:fout, off:off + cl],
                            rbt[ci][0:cl, b * fout:(b + 1) * fout],
                            ident[0:cl, 0:cl], is_transpose=True,
                            start=(j == 0), stop=(j == len(blks) - 1))
                    sl = slice(bk * 512, (bk + 1) * 512)
                    nc.vector.tensor_tensor(preact[0:fout, sl],
                                            ut[0:fout, :], p0a[0:fout, sl],
                                            mybir.AluOpType.add)
                return preact

            def cell(cname, hf, xh_t, xh_next):
                """DCGRU cell on one batch-half. x rows 64:128, h rows 0:64."""
                din = CELL_DIN[cname]
                xr = xrh[hf]
                r_t, u_t, c_t = r_h[hf], u_h[hf], c_h[hf]
                pre_g = gconv(cname, xh_t, "g", 128, hf)
                gb = wt[f"{cname}_gb"]
                for ci in range(3):
                    sl = slice(ci * 512, (ci + 1) * 512)
                    nc.scalar.activation(r_t[:, sl], pre_g[0:64, sl],
                                         AFT.Sigmoid, bias=gb[0:64, 0:1])
                    nc.scalar.activation(u_t[:, sl], pre_g[64:128, sl],
                                         AFT.Sigmoid, bias=gb[64:128, 0:1])
                nc.vector.tensor_tensor(xr[0:64, :], r_t[:, :], xh_t[0:64, :],
                                        mybir.AluOpType.mult)
                nc.vector.tensor_copy(xr[64:64 + din, :],
                                      xh_t[64:64 + din, :])
                pre_c = gconv(cname, xr, "c", 64, hf)
                cb = wt[f"{cname}_cb"]
                for ci in range(3):
                    sl = slice(ci * 512, (ci + 1) * 512)
                    nc.scalar.activation(c_t[:, sl], pre_c[0:64, sl],
                                         AFT.Tanh, bias=cb[0:64, 0:1])
                # h' = c + u*(h-c); r_t is dead, reuse it for temps
                nc.vector.tensor_tensor(r_t[:, :], xh_t[0:64, :], c_t[:, :],
                                        mybir.AluOpType.subtract)
                nc.vector.tensor_tensor(r_t[:, :], u_t[:, :], r_t[:, :],
                                        mybir.AluOpType.mult)
                nc.vector.tensor_tensor(xh_t[0:64, :], c_t[:, :], r_t[:, :],
                                        mybir.AluOpType.add)
                if xh_next is not None:
                    nc.vector.tensor_copy(xh_next[64:128, :], xh_t[0:64, :])

            # ---- encoder ----
            xr3 = x_in[:].rearrange("t d (g f) -> t d g f", g=2)
            for t in range(nsteps_enc):
                for hf in range(2):
                    nc.sync.dma_start(out=xh[("enc0", hf)][64:66, :],
                                      in_=xr3[t, :, hf, :])
                for hf in range(2):
                    cell("enc0", hf, xh[("enc0", hf)], xh[("enc1", hf)])
                    cell("enc1", hf, xh[("enc1", hf)], None)

            # ---- copy encoder state to decoder ----
            for hf in range(2):
                nc.gpsimd.tensor_copy(xh[("dec0", hf)][0:64, :],
                                      xh[("enc0", hf)][0:64, :])
                nc.gpsimd.tensor_copy(xh[("dec1", hf)][0:64, :],
                                      xh[("enc1", hf)][0:64, :])

            # ---- decoder ----
            for t in range(nsteps_dec):
                for hf in range(2):
                    cell("dec0", hf, xh[("dec0", hf)], xh[("dec1", hf)])
                    cell("dec1", hf, xh[("dec1", hf)], None)
                    for ci in range(3):
                        sl = slice(ci * 512, (ci + 1) * 512)
                        pt = psp.tile([128, 512], F32, tag="pstage")
                        nc.tensor.matmul(pt[0:1, :], wt["pW"][0:64, :],
                                         xh[("dec1", hf)][0:64, sl],
                                         start=True, stop=True)
                        nc.scalar.activation(xh[("dec0", hf)][64:65, sl],
                                             pt[0:1, :], AFT.Identity,
                                             bias=wt["pb"][0:1, 0:1])
                    ov = xh[("dec0", hf)][64:65, :].bitcast(F32).rearrange(
                        "p (b n) -> p b n", b=HB)
                    nc.sync.dma_start(out=out_d[t][:, hf * HB:(hf + 1) * HB, :],
                                      in_=ov[:, :, 0:N])

    nc.finalize()
    _BUILD_CACHE[key] = nc
    return nc


def _prep_inputs(inputs, support, weights):
    """Host-side prep. Returns (shared_map, per_core_x list)."""
    shared = {"s": np.ascontiguousarray(support, np.float32)}
    for c in CELLS:
        din = CELL_DIN[c]
        ga0, gw1, gw2 = _pad_w(weights[f"{c}_gate_W"], din, 2 * U)
        ca0, cw1, cw2 = _pad_w(weights[f"{c}_cand_W"], din, U)
        gb = np.zeros((128, 1), np.float32)
        gb[:, 0] = weights[f"{c}_gate_b"]
        cb = np.zeros((64, 1), np.float32)
        cb[:, 0] = weights[f"{c}_cand_b"]
        shared.update({f"{c}_gA0": ga0, f"{c}_gW1": gw1, f"{c}_gW2": gw2,
                       f"{c}_cA0": ca0, f"{c}_cW1": cw1, f"{c}_cW2": cw2,
                       f"{c}_gb": gb, f"{c}_cb": cb})
    shared["pW"] = np.ascontiguousarray(weights["proj_W"], np.float32)
    shared["pb"] = np.asarray(weights["proj_b"], np.float32).reshape(1, 1)

    # inputs (T, B, N*DIN) -> per-core (T, DIN, AF) with node padding
    x = np.asarray(inputs, np.float32).reshape(T, B, N, DIN)
    per_core = []
    for c in range(NCORES):
        xc = x[:, c * BL:(c + 1) * BL]                  # (T, BL, N, DIN)
        xp = np.zeros((T, DIN, BL, NB), np.float32)
        xp[:, :, :, 0:N] = xc.transpose(0, 3, 1, 2)
        per_core.append(xp.reshape(T, DIN, AF))
    return shared, per_core


def kernel(**inputs) -> np.ndarray:
    support = np.asarray(inputs["support"], np.float32)
    weights = {k: np.asarray(v, np.float32) for k, v in inputs.items()
               if k not in ("inputs", "support")}
    shared, per_core_x = _prep_inputs(inputs["inputs"], support, weights)

    nc = _build(T, HZ)
    if os.environ.get("DCRNN_TRACE"):
        _install_ntff_hook()
    in_maps = [dict(shared, x=per_core_x[c]) for c in range(NCORES)]
    res = run_bass_kernel_spmd(nc, in_maps, list(range(NCORES)),
                               trace=bool(os.environ.get("DCRNN_TRACE")))
    global LAST_RESULT
    LAST_RESULT = res
    if res.exec_time_ns is not None:
        print(f"HW exec time: {res.exec_time_ns} ns")
    outs = [res.results[c]["out"].reshape(HZ, BL, N) for c in range(NCORES)]
    return np.concatenate(outs, axis=1).astype(np.float32)


if __name__ == "__main__":
    sys.path.insert(0, "/root/problem")
    import reference
    ins = reference.setup_inputs()
    ins = {k: np.asarray(v) for k, v in ins.items()}
    exp = np.asarray(reference.reference(**ins))
    act = kernel(**ins)
    err = np.max(np.abs(act - exp)) / (np.abs(exp).max() + 1e-30)
    print("Relative error:", err)



# revision 8
# speedup vs baseline: 2.0207x; 2.0207x over previous
"""DCRNN (PEMS-BAY) Trainium2 Bass kernel, data-parallel over batch on 8 cores.

Transpose-free gconv via S^2 precompute, fp16 matmuls/states, fp32 psum.

Layouts per core (local batch BL=8, split in 2 halves of HB=4):
  A1: [feature partitions, b*384 + n]   (state tiles XH/XR: rows 0:64 = h|rh,
      rows 64:64+din = x)
  B:  [node-chunk partitions (128/128/69), b*Fout + f]  (W-product tiles)
gconv:  pre = X@A0 + S@(X@W1) + S^2@(X@(2*W2))      [A0 = W0 - W2]
  P2B/P1B = direct-to-B W-matmuls (lhsT = XH col-slice, rhs = weight);
  preact accumulated per batch in one psum bank: P0 (lhsT=A0, rhs=XH)
  start=True, then S@P1B + S2@P2B with S/S2 as *rhs* (lhsT = P_B chunk)
  which lands back in A1 layout.  No PE transposes anywhere.
Cand gconv packs 2 batches into 128 partitions ((b%2)*64+f) per psum bank.
"""
import sys
import os
import numpy as np

sys.path.insert(0, "/opt/trn_rl_repo")

import concourse.bass as bass  # noqa: E402
import concourse.mybir as mybir  # noqa: E402
import concourse.tile as tile  # noqa: E402
from concourse import bacc  # noqa: E402
from concourse.bass_utils import run_bass_kernel_spmd  # noqa: E402

# problem constants
N = 325
B = 64
T = 12
HZ = 12
U = 64
DIN = 2
DOUT = 1
NCORES = 8
BL = B // NCORES          # 8 local batch
NB = 384                  # padded node stride per batch
AF = BL * NB              # 3072 A-layout free width
NCH = [(0, 128), (128, 128), (256, 69)]   # node chunks (offset, len)
HB = BL // 2              # 4 batches per half
HAF = HB * NB             # 1536 A cols per half

F32 = mybir.dt.float32
F16 = mybir.dt.float16
AFT = mybir.ActivationFunctionType
ALU = mybir.AluOpType

CELLS = ["enc0", "enc1", "dec0", "dec1"]
CELL_DIN = {"enc0": DIN, "enc1": U, "dec0": DOUT, "dec1": U}

_BUILD_CACHE = {}
LAST_RESULT = None


def _install_ntff_hook():
    """Register the axon NTFF profiling hook if the image lacks antenv.axon_hooks."""
    import types
    import antenv
    if getattr(antenv, "axon_hooks", None) is not None:
        return
    m = types.ModuleType("antenv.axon_hooks")
    state = {"h": None}
    m.set_axon_ntff_profile_hook = lambda h: state.__setitem__("h", h)
    m.get_axon_ntff_profile_hook = lambda: state["h"]
    sys.modules["antenv.axon_hooks"] = m
    antenv.axon_hooks = m
    try:
        from trn_agent_boot.trn_boot import _ntff_profile_via_ctypes
        hook = _ntff_profile_via_ctypes("/opt/axon/libaxon_pjrt.so")
        if hook is not None:
            m.set_axon_ntff_profile_hook(hook)
    except Exception:
        pass


def _pad_w(w, din, fout):
    """(3F, fout) -> three [128, fout] fp16 mats A0, W1, 2*W2.

    Padded row map: rows 0:64 <- h/rh features (orig rows din:F),
    rows 64:64+din <- x features (orig rows 0:din). Others zero.
    """
    f = din + U
    w0, w1, w2 = w[0:f], w[f:2 * f], w[2 * f:3 * f]

    def pad(m):
        p = np.zeros((128, fout), np.float32)
        p[0:64] = m[din:f]
        p[64:64 + din] = m[0:din]
        return p.astype(np.float16)

    return pad(w0 - w2), pad(w1), pad(2.0 * w2)


def _build(nsteps_enc, nsteps_dec):
    key = (nsteps_enc, nsteps_dec)
    if key in _BUILD_CACHE:
        return _BUILD_CACHE[key]

    nc = bacc.Bacc()
    # ---- DRAM params ----
    x_in = nc.declare_dram_parameter("x", [T, DIN, AF], F16, isOutput=False)
    s_in = nc.declare_dram_parameter("s", [3, 128, N], F16, isOutput=False)
    s2_in = nc.declare_dram_parameter("s2", [3, 128, N], F16, isOutput=False)
    wparams = {}
    for c in CELLS:
        for nm, shp, dt_ in [("gA0", [128, 128], F16), ("gW1", [128, 128], F16),
                             ("gW2", [128, 128], F16), ("cA0", [128, 64], F16),
                             ("cW1", [128, 64], F16), ("cW2", [128, 64], F16),
                             ("gb", [128, 1], F32), ("cb2", [128, 1], F32)]:
            wparams[f"{c}_{nm}"] = nc.declare_dram_parameter(
                f"{c}_{nm}", shp, dt_, isOutput=False)
    wparams["pW"] = nc.declare_dram_parameter("pW", [64, 1], F16, isOutput=False)
    wparams["pb128"] = nc.declare_dram_parameter("pb128", [128, 1], F32,
                                                 isOutput=False)
    out_d = nc.declare_dram_parameter("out", [HZ, 1, BL, N], F32, isOutput=True)

    with tile.TileContext(nc) as tc:
        with tc.tile_pool(name="const", bufs=1) as cp, \
             tc.tile_pool(name="state", bufs=1) as st, \
             tc.tile_pool(name="bprod", bufs=1) as bp, \
             tc.tile_pool(name="pstage", bufs=3, space="PSUM") as psp, \
             tc.tile_pool(name="pgate", bufs=3, space="PSUM") as pgp, \
             tc.tile_pool(name="pcand", bufs=2, space="PSUM") as pcp:

            # ---- constants to SBUF ----
            wt = {}
            for c in CELLS:
                for nm in ["gA0", "gW1", "gW2"]:
                    wt[f"{c}_{nm}"] = cp.tile([128, 128], F16, tag=f"{c}_{nm}",
                                              name=f"{c}_{nm}")
                for nm in ["cA0", "cW1", "cW2"]:
                    wt[f"{c}_{nm}"] = cp.tile([128, 64], F16, tag=f"{c}_{nm}",
                                              name=f"{c}_{nm}")
                wt[f"{c}_gb"] = cp.tile([128, 1], F32, tag=f"{c}_gb",
                                        name=f"{c}_gb")
                wt[f"{c}_cb2"] = cp.tile([128, 1], F32, tag=f"{c}_cb2",
                                         name=f"{c}_cb2")
            wt["pW"] = cp.tile([64, 1], F16, tag="pW", name="pW")
            wt["pb128"] = cp.tile([128, 1], F32, tag="pb128", name="pb128")
            for k, t in wt.items():
                nc.sync.dma_start(out=t, in_=wparams[k][:])
            s_t, s2_t = [], []
            for ci, (c0, cl) in enumerate(NCH):
                stl = cp.tile([128, N], F16, tag=f"s{ci}", name=f"s{ci}")
                nc.sync.dma_start(out=stl[0:cl, :], in_=s_in[ci, 0:cl, :])
                s_t.append(stl)
                s2l = cp.tile([128, N], F16, tag=f"s2{ci}", name=f"s2{ci}")
                nc.sync.dma_start(out=s2l[0:cl, :], in_=s2_in[ci, 0:cl, :])
                s2_t.append(s2l)

            # ---- state tiles ----
            xh, xr = {}, {}
            ru, ct, uu = {}, {}, {}
            for hf in range(2):
                for c in CELLS:
                    xh[(c, hf)] = st.tile([128, HAF], F16, tag=f"xh_{c}_{hf}",
                                          name=f"xh_{c}_{hf}")
                    xr[(c, hf)] = st.tile([128, HAF], F16, tag=f"xr_{c}_{hf}",
                                          name=f"xr_{c}_{hf}")
                ru[hf] = st.tile([128, HAF], F16, tag=f"ru{hf}", name=f"ru{hf}")
                ct[hf] = st.tile([64, HAF], F16, tag=f"ct{hf}", name=f"ct{hf}")
                uu[hf] = st.tile([64, HAF], F16, tag=f"uu{hf}", name=f"uu{hf}")

            projf = {hf: st.tile([1, HAF], F32, tag=f"projf{hf}",
                                 name=f"projf{hf}") for hf in range(2)}

            for tl in list(xh.values()) + list(xr.values()):
                nc.vector.memset(tl[:, :], 0.0)
            tc.strict_bb_all_engine_barrier()

            def wprod(cell, hf, src_t, wprefix, fout):
                """Direct-to-B W-matmuls: out[(n-chunk), b*fout+f] tiles.

                Returns 3 SBUF fp16 tiles [cl, HB*fout]. evac engines:
                W2-product on Act, W1-product on Pool.
                """
                w2 = wt[f"{cell}_{wprefix}W2"]
                w1 = wt[f"{cell}_{wprefix}W1"]
                fw = HB * fout
                out = {}
                for role, w_ in (("p2", w2), ("p1", w1)):
                    tiles = []
                    for ci, (c0, cl) in enumerate(NCH):
                        stg = psp.tile([128, 512], F32, tag="stage")
                        for b in range(HB):
                            nc.tensor.matmul(
                                stg[0:cl, b * fout:(b + 1) * fout],
                                src_t[:, b * NB + c0: b * NB + c0 + cl],
                                w_[0:128, 0:fout],
                                start=(b == 0), stop=(b == HB - 1))
                        dst = bp.tile([128, fw], F16,
                                      tag=f"{cell[:1]}{wprefix}{role}{ci}_{hf}",
                                      name=f"{wprefix}{role}{ci}_{hf}")
                        if role == "p2":
                            nc.scalar.copy(dst[0:cl, 0:fw], stg[0:cl, 0:fw])
                        else:
                            nc.vector.tensor_copy(dst[0:cl, 0:fw],
                                                  stg[0:cl, 0:fw])
                        tiles.append(dst)
                    out[role] = tiles
                return out["p2"], out["p1"]

            def cell(cname, hf):
                """One DCGRU cell on one batch-half."""
                xh_t = xh[(cname, hf)]
                xr_t = xr[(cname, hf)]
                gb = wt[f"{cname}_gb"]
                cb2 = wt[f"{cname}_cb2"]
                ru_t, c_t = ru[hf], ct[hf]

                # ---- gate gconv (fout=128) ----
                p2b, p1b = wprod(cname, hf, xh_t, "g", 128)
                a0 = wt[f"{cname}_gA0"]
                for b in range(HB):
                    pg = pgp.tile([128, 384], F32, tag="pgb")
                    nc.tensor.matmul(pg[0:128, 0:384], a0[0:128, 0:128],
                                     xh_t[:, b * NB:(b + 1) * NB],
                                     start=True, stop=False)
                    for pt, srcs in ((p1b, s_t), (p2b, s2_t)):
                        for k, (c0, cl) in enumerate(NCH):
                            last = pt is p2b and k == 2
                            nc.tensor.matmul(
                                pg[0:128, 0:N],
                                pt[k][0:cl, b * 128:(b + 1) * 128],
                                srcs[k][0:cl, 0:N],
                                start=False, stop=last)
                    nc.scalar.activation(ru_t[:, b * NB:(b + 1) * NB],
                                         pg[0:128, 0:384], AFT.Sigmoid,
                                         bias=gb[0:128, 0:1])
                # rh -> xr rows 0:64; u down to base-0 tile for the GRU
                nc.vector.tensor_tensor(xr_t[0:64, :], ru_t[0:64, :],
                                        xh_t[0:64, :], ALU.mult)
                nc.vector.tensor_copy(uu[hf][0:64, :], ru_t[64:128, :])

                # ---- cand gconv (fout=64, 2-batch packed psum) ----
                q2b, q1b = wprod(cname, hf, xr_t, "c", 64)
                ca0 = wt[f"{cname}_cA0"]
                for pr in range(2):
                    pc = pcp.tile([128, 384], F32, tag="pcb")
                    first = True
                    for pt, srcs in ((q1b, s_t), (q2b, s2_t)):
                        for k, (c0, cl) in enumerate(NCH):
                            nc.tensor.matmul(
                                pc[0:128, 0:N],
                                pt[k][0:cl, pr * 128:(pr + 1) * 128],
                                srcs[k][0:cl, 0:N],
                                start=first, stop=False)
                            first = False
                    for par in range(2):
                        b = 2 * pr + par
                        nc.tensor.matmul(
                            pc[par * 64:(par + 1) * 64, 0:384],
                            ca0[0:128, 0:64],
                            xr_t[:, b * NB:(b + 1) * NB],
                            start=False, stop=(par == 1))
                    for par in range(2):
                        b = 2 * pr + par
                        nc.scalar.activation(
                            c_t[0:64, b * NB:(b + 1) * NB],
                            pc[par * 64:(par + 1) * 64, 0:384], AFT.Tanh,
                            bias=cb2[par * 64:(par + 1) * 64, 0:1])

                # ---- GRU update (A1, fp16 on DVE) ----
                # d = h - c -> xr[0:64] (rh dead); m = u*d; h' = c + m
                nc.vector.tensor_tensor(xr_t[0:64, :], xh_t[0:64, :],
                                        c_t[0:64, :], ALU.subtract)
                nc.vector.tensor_tensor(xr_t[0:64, :], uu[hf][0:64, :],
                                        xr_t[0:64, :], ALU.mult)
                nc.vector.tensor_tensor(xh_t[0:64, :], c_t[0:64, :],
                                        xr_t[0:64, :], ALU.add)

            def mirror(src_cell, dst_cell, hf):
                """Copy h' of src layer into x rows of dst layer tiles."""
                nc.vector.tensor_copy(xh[(dst_cell, hf)][64:128, :],
                                      xh[(src_cell, hf)][0:64, :])
                nc.vector.tensor_copy(xr[(dst_cell, hf)][64:128, :],
                                      xh[(src_cell, hf)][0:64, :])

            # ---- encoder ----
            xr3 = x_in[:].rearrange("t d (g f) -> t d g f", g=2)
            for t in range(nsteps_enc):
                for hf in range(2):
                    nc.sync.dma_start(out=xh[("enc0", hf)][64:66, :],
                                      in_=xr3[t, :, hf, :])
                    nc.sync.dma_start(out=xr[("enc0", hf)][64:66, :],
                                      in_=xr3[t, :, hf, :])
                for hf in range(2):
                    cell("enc0", hf)
                    mirror("enc0", "enc1", hf)
                    cell("enc1", hf)

            # ---- copy encoder state to decoder ----
            for hf in range(2):
                nc.vector.tensor_copy(xh[("dec0", hf)][0:64, :],
                                      xh[("enc0", hf)][0:64, :])
                nc.vector.tensor_copy(xh[("dec1", hf)][0:64, :],
                                      xh[("enc1", hf)][0:64, :])

            # ---- decoder ----
            for t in range(nsteps_dec):
                for hf in range(2):
                    cell("dec0", hf)
                    mirror("dec0", "dec1", hf)
                    cell("dec1", hf)
                    # projection: out = h_dec1 @ pW + pb
                    pf = projf[hf]
                    for b in range(HB):
                        pp = psp.tile([1, 384], F32, tag="stage")
                        nc.tensor.matmul(
                            pp[0:1, 0:384],
                            wt["pW"][0:64, 0:1],
                            xh[("dec1", hf)][0:64, b * NB:(b + 1) * NB],
                            start=True, stop=True)
                        nc.scalar.activation(
                            pf[0:1, b * NB:(b + 1) * NB],
                            pp[0:1, 0:384], AFT.Identity,
                            bias=wt["pb128"][0:1, 0:1])
                    ov = pf[0:1, :].rearrange("p (b n) -> p b n", b=HB)
                    nc.sync.dma_start(out=out_d[t][:, hf * HB:(hf + 1) * HB, :],
                                      in_=ov[:, :, 0:N])
                    if t < nsteps_dec - 1:
                        # feedback: proj -> x rows of dec0
                        for b in range(HB):
                            nc.vector.tensor_copy(
                                xh[("dec0", hf)][64:65, b * NB:(b + 1) * NB],
                                pf[0:1, b * NB:(b + 1) * NB])
                        nc.vector.tensor_copy(xr[("dec0", hf)][64:65, :],
                                              xh[("dec0", hf)][64:65, :])

    nc.finalize()
    _BUILD_CACHE[key] = nc
    return nc


def _prep_inputs(inputs, support, weights):
    """Host-side prep. Returns (shared_map, per_core_x list)."""
    s32 = np.asarray(support, np.float32)
    s2_32 = s32 @ s32
    shared = {}
    for nm, m in (("s", s32), ("s2", s2_32)):
        chunks = np.zeros((3, 128, N), np.float16)
        for ci, (c0, cl) in enumerate(NCH):
            chunks[ci, 0:cl, :] = m[c0:c0 + cl, :].astype(np.float16)
        shared[nm] = chunks
    for c in CELLS:
        din = CELL_DIN[c]
        ga0, gw1, gw2 = _pad_w(weights[f"{c}_gate_W"], din, 2 * U)
        ca0, cw1, cw2 = _pad_w(weights[f"{c}_cand_W"], din, U)
        gb = np.zeros((128, 1), np.float32)
        gb[:, 0] = weights[f"{c}_gate_b"]
        cb2 = np.zeros((128, 1), np.float32)
        cb2[0:64, 0] = weights[f"{c}_cand_b"]
        cb2[64:128, 0] = weights[f"{c}_cand_b"]
        shared.update({f"{c}_gA0": ga0, f"{c}_gW1": gw1, f"{c}_gW2": gw2,
                       f"{c}_cA0": ca0, f"{c}_cW1": cw1, f"{c}_cW2": cw2,
                       f"{c}_gb": gb, f"{c}_cb2": cb2})
    shared["pW"] = np.ascontiguousarray(weights["proj_W"]).astype(np.float16)
    pb128 = np.zeros((128, 1), np.float32)
    pb128[:, 0] = float(np.asarray(weights["proj_b"]).reshape(-1)[0])
    shared["pb128"] = pb128

    # inputs (T, B, N*DIN) -> per-core (T, DIN, AF) with node padding
    x = np.asarray(inputs, np.float32).reshape(T, B, N, DIN)
    per_core = []
    for c in range(NCORES):
        xc = x[:, c * BL:(c + 1) * BL]                  # (T, BL, N, DIN)
        xp = np.zeros((T, DIN, BL, NB), np.float16)
        xp[:, :, :, 0:N] = xc.transpose(0, 3, 1, 2)
        per_core.append(xp.reshape(T, DIN, AF))
    return shared, per_core


def kernel(**inputs) -> np.ndarray:
    support = np.asarray(inputs["support"], np.float32)
    weights = {k: np.asarray(v, np.float32) for k, v in inputs.items()
               if k not in ("inputs", "support")}
    shared, per_core_x = _prep_inputs(inputs["inputs"], support, weights)

    nc = _build(T, HZ)
    if os.environ.get("DCRNN_TRACE"):
        _install_ntff_hook()
    in_maps = [dict(shared, x=per_core_x[c]) for c in range(NCORES)]
    res = run_bass_kernel_spmd(nc, in_maps, list(range(NCORES)),
                               trace=bool(os.environ.get("DCRNN_TRACE")))
    global LAST_RESULT
    LAST_RESULT = res
    if res.exec_time_ns is not None:
        print(f"HW exec time: {res.exec_time_ns} ns")
    outs = [res.results[c]["out"].reshape(HZ, BL, N) for c in range(NCORES)]
    return np.concatenate(outs, axis=1).astype(np.float32)


if __name__ == "__main__":
    sys.path.insert(0, "/root/problem")
    import reference
    ins = reference.setup_inputs()
    ins = {k: np.asarray(v) for k, v in ins.items()}
    exp = np.asarray(reference.reference(**ins))
    act = kernel(**ins)
    err = np.max(np.abs(act - exp)) / (np.abs(exp).max() + 1e-30)
    print("Relative error:", err)


# revision 11
# speedup vs baseline: 2.0686x; 1.0237x over previous
"""DCRNN (PEMS-BAY) Trainium2 Bass kernel, data-parallel over batch on 8 cores.

Transpose-free gconv via S^2 precompute, fp16 matmuls/states, fp32 psum.

Layouts per core (local batch BL=8, split in 2 halves of HB=4):
  A1: [feature partitions, b*384 + n]   (state tiles XH/XR: rows 0:64 = h|rh,
      rows 64:64+din = x)
  B:  [node-chunk partitions (128/128/69), b*Fout + f]  (W-product tiles)
gconv:  pre = X@A0 + S@(X@W1) + S^2@(X@(2*W2))      [A0 = W0 - W2]
  P2B/P1B = direct-to-B W-matmuls (lhsT = XH col-slice, rhs = weight);
  preact accumulated per batch in one psum bank: P0 (lhsT=A0, rhs=XH)
  start=True, then S@P1B + S2@P2B with S/S2 as *rhs* (lhsT = P_B chunk)
  which lands back in A1 layout.  No PE transposes anywhere.
Cand gconv packs 2 batches into 128 partitions ((b%2)*64+f) per psum bank.
"""
import sys
import os
import numpy as np

sys.path.insert(0, "/opt/trn_rl_repo")

import concourse.bass as bass  # noqa: E402
import concourse.mybir as mybir  # noqa: E402
import concourse.tile as tile  # noqa: E402
from concourse import bacc  # noqa: E402
from concourse.bass_utils import run_bass_kernel_spmd  # noqa: E402

# problem constants
N = 325
B = 64
T = 12
HZ = 12
U = 64
DIN = 2
DOUT = 1
NCORES = 8
BL = B // NCORES          # 8 local batch
NB = 384                  # padded node stride per batch
AF = BL * NB              # 3072 A-layout free width
NCH = [(0, 128), (128, 128), (256, 69)]   # node chunks (offset, len)
HB = BL // 2              # 4 batches per half
HAF = HB * NB             # 1536 A cols per half

F32 = mybir.dt.float32
F16 = mybir.dt.float16
AFT = mybir.ActivationFunctionType
ALU = mybir.AluOpType

CELLS = ["enc0", "enc1", "dec0", "dec1"]
CELL_DIN = {"enc0": DIN, "enc1": U, "dec0": DOUT, "dec1": U}

_BUILD_CACHE = {}
LAST_RESULT = None


def _install_ntff_hook():
    """Register the axon NTFF profiling hook if the image lacks antenv.axon_hooks."""
    import types
    import antenv
    if getattr(antenv, "axon_hooks", None) is not None:
        return
    m = types.ModuleType("antenv.axon_hooks")
    state = {"h": None}
    m.set_axon_ntff_profile_hook = lambda h: state.__setitem__("h", h)
    m.get_axon_ntff_profile_hook = lambda: state["h"]
    sys.modules["antenv.axon_hooks"] = m
    antenv.axon_hooks = m
    try:
        from trn_agent_boot.trn_boot import _ntff_profile_via_ctypes
        hook = _ntff_profile_via_ctypes("/opt/axon/libaxon_pjrt.so")
        if hook is not None:
            m.set_axon_ntff_profile_hook(hook)
    except Exception:
        pass


def _pad_w(w, din, fout):
    """(3F, fout) -> three [128, fout] fp16 mats A0, W1, 2*W2.

    Padded row map: rows 0:64 <- h/rh features (orig rows din:F),
    rows 64:64+din <- x features (orig rows 0:din). Others zero.
    """
    f = din + U
    w0, w1, w2 = w[0:f], w[f:2 * f], w[2 * f:3 * f]

    def pad(m):
        p = np.zeros((128, fout), np.float32)
        p[0:64] = m[din:f]
        p[64:64 + din] = m[0:din]
        return p.astype(np.float16)

    return pad(w0 - w2), pad(w1), pad(2.0 * w2)


def _build(nsteps_enc, nsteps_dec):
    key = (nsteps_enc, nsteps_dec)
    if key in _BUILD_CACHE:
        return _BUILD_CACHE[key]

    nc = bacc.Bacc()
    # ---- DRAM params ----
    x_in = nc.declare_dram_parameter("x", [T, DIN, AF], F16, isOutput=False)
    s_in = nc.declare_dram_parameter("s", [3, 128, N], F16, isOutput=False)
    s2_in = nc.declare_dram_parameter("s2", [3, 128, N], F16, isOutput=False)
    wparams = {}
    for c in CELLS:
        for nm, shp, dt_ in [("gA0", [128, 128], F16), ("gW1", [128, 128], F16),
                             ("gW2", [128, 128], F16), ("cA0", [128, 64], F16),
                             ("cW1", [128, 64], F16), ("cW2", [128, 64], F16),
                             ("gb", [128, 1], F32), ("cb2", [128, 1], F32)]:
            wparams[f"{c}_{nm}"] = nc.declare_dram_parameter(
                f"{c}_{nm}", shp, dt_, isOutput=False)
    wparams["pW"] = nc.declare_dram_parameter("pW", [64, 1], F16, isOutput=False)
    wparams["pb128"] = nc.declare_dram_parameter("pb128", [128, 1], F32,
                                                 isOutput=False)
    out_d = nc.declare_dram_parameter("out", [HZ, 1, BL, N], F32, isOutput=True)

    with tile.TileContext(nc) as tc:
        with tc.tile_pool(name="const", bufs=1) as cp, \
             tc.tile_pool(name="state", bufs=1) as st, \
             tc.tile_pool(name="bprod", bufs=1) as bp, \
             tc.tile_pool(name="pstage", bufs=3, space="PSUM") as psp, \
             tc.tile_pool(name="pgate", bufs=3, space="PSUM") as pgp, \
             tc.tile_pool(name="pcand", bufs=2, space="PSUM") as pcp:

            # ---- constants to SBUF ----
            wt = {}
            for c in CELLS:
                for nm in ["gA0", "gW1", "gW2"]:
                    wt[f"{c}_{nm}"] = cp.tile([128, 128], F16, tag=f"{c}_{nm}",
                                              name=f"{c}_{nm}")
                for nm in ["cA0", "cW1", "cW2"]:
                    wt[f"{c}_{nm}"] = cp.tile([128, 64], F16, tag=f"{c}_{nm}",
                                              name=f"{c}_{nm}")
                wt[f"{c}_gb"] = cp.tile([128, 1], F32, tag=f"{c}_gb",
                                        name=f"{c}_gb")
                wt[f"{c}_cb2"] = cp.tile([128, 1], F32, tag=f"{c}_cb2",
                                         name=f"{c}_cb2")
            wt["pW"] = cp.tile([64, 1], F16, tag="pW", name="pW")
            wt["pb128"] = cp.tile([128, 1], F32, tag="pb128", name="pb128")
            for k, t in wt.items():
                nc.sync.dma_start(out=t, in_=wparams[k][:])
            s_t, s2_t = [], []
            for ci, (c0, cl) in enumerate(NCH):
                stl = cp.tile([128, N], F16, tag=f"s{ci}", name=f"s{ci}")
                nc.sync.dma_start(out=stl[0:cl, :], in_=s_in[ci, 0:cl, :])
                s_t.append(stl)
                s2l = cp.tile([128, N], F16, tag=f"s2{ci}", name=f"s2{ci}")
                nc.sync.dma_start(out=s2l[0:cl, :], in_=s2_in[ci, 0:cl, :])
                s2_t.append(s2l)

            # ---- state tiles ----
            xh, xr = {}, {}
            ru, ct, uu = {}, {}, {}
            for hf in range(2):
                for c in CELLS:
                    xh[(c, hf)] = st.tile([128, HAF], F16, tag=f"xh_{c}_{hf}",
                                          name=f"xh_{c}_{hf}")
                    xr[(c, hf)] = st.tile([128, HAF], F16, tag=f"xr_{c}_{hf}",
                                          name=f"xr_{c}_{hf}")
                ru[hf] = st.tile([128, HAF], F16, tag=f"ru{hf}", name=f"ru{hf}")
                ct[hf] = st.tile([64, HAF], F16, tag=f"ct{hf}", name=f"ct{hf}")
                uu[hf] = st.tile([64, HAF], F16, tag=f"uu{hf}", name=f"uu{hf}")

            projf = {hf: st.tile([1, HAF], F32, tag=f"projf{hf}",
                                 name=f"projf{hf}") for hf in range(2)}

            for tl in (list(xh.values()) + list(xr.values())
                       + list(ru.values()) + list(ct.values())
                       + list(uu.values())):
                nc.vector.memset(tl[:, :], 0.0)
            tc.strict_bb_all_engine_barrier()

            def wprod(cell, hf, src_t, wprefix, fout):
                """Direct-to-B W-matmuls: out[(n-chunk), b*fout+f] tiles.

                Returns 3 SBUF fp16 tiles [cl, HB*fout]. evac engines:
                W2-product on Act, W1-product on Pool.
                """
                w2 = wt[f"{cell}_{wprefix}W2"]
                w1 = wt[f"{cell}_{wprefix}W1"]
                fw = HB * fout
                out = {}
                for role, w_ in (("p2", w2), ("p1", w1)):
                    tiles = []
                    for ci, (c0, cl) in enumerate(NCH):
                        stg = psp.tile([128, 512], F32, tag="stage")
                        for b in range(HB):
                            nc.tensor.matmul(
                                stg[0:cl, b * fout:(b + 1) * fout],
                                src_t[:, b * NB + c0: b * NB + c0 + cl],
                                w_[0:128, 0:fout],
                                start=(b == 0), stop=(b == HB - 1))
                        dst = bp.tile([128, fw], F16,
                                      tag=f"{cell[:1]}{wprefix}{role}{ci}_{hf}",
                                      name=f"{wprefix}{role}{ci}_{hf}")
                        if role == "p2":
                            nc.scalar.copy(dst[0:cl, 0:fw], stg[0:cl, 0:fw])
                        else:
                            nc.vector.tensor_copy(dst[0:cl, 0:fw],
                                                  stg[0:cl, 0:fw])
                        tiles.append(dst)
                    out[role] = tiles
                return out["p2"], out["p1"]

            def cell(cname, hf):
                """One DCGRU cell on one batch-half."""
                xh_t = xh[(cname, hf)]
                xr_t = xr[(cname, hf)]
                gb = wt[f"{cname}_gb"]
                cb2 = wt[f"{cname}_cb2"]
                ru_t, c_t = ru[hf], ct[hf]

                # ---- gate gconv (fout=128) ----
                p2b, p1b = wprod(cname, hf, xh_t, "g", 128)
                a0 = wt[f"{cname}_gA0"]
                for b in range(HB):
                    pg = pgp.tile([128, 384], F32, tag="pgb")
                    nc.tensor.matmul(pg[0:128, 0:N], a0[0:128, 0:128],
                                     xh_t[:, b * NB:b * NB + N],
                                     start=True, stop=False)
                    for pt, srcs in ((p1b, s_t), (p2b, s2_t)):
                        for k, (c0, cl) in enumerate(NCH):
                            last = pt is p2b and k == 2
                            nc.tensor.matmul(
                                pg[0:128, 0:N],
                                pt[k][0:cl, b * 128:(b + 1) * 128],
                                srcs[k][0:cl, 0:N],
                                start=False, stop=last)
                    nc.scalar.activation(ru_t[:, b * NB:b * NB + N],
                                         pg[0:128, 0:N], AFT.Sigmoid,
                                         bias=gb[0:128, 0:1])
                    # rh -> xr rows 0:64 (per batch, pipelines with sigmoid)
                    nc.vector.tensor_tensor(
                        xr_t[0:64, b * NB:(b + 1) * NB],
                        ru_t[0:64, b * NB:(b + 1) * NB],
                        xh_t[0:64, b * NB:(b + 1) * NB], ALU.mult)
                # u down to a base-0 tile for the GRU (Act has slack here)
                nc.scalar.copy(uu[hf][0:64, :], ru_t[64:128, :])

                # ---- cand gconv (fout=64, 2-batch packed psum) ----
                q2b, q1b = wprod(cname, hf, xr_t, "c", 64)
                ca0 = wt[f"{cname}_cA0"]
                for pr in range(2):
                    pc = pcp.tile([128, 384], F32, tag="pcb")
                    first = True
                    for pt, srcs in ((q1b, s_t), (q2b, s2_t)):
                        for k, (c0, cl) in enumerate(NCH):
                            nc.tensor.matmul(
                                pc[0:128, 0:N],
                                pt[k][0:cl, pr * 128:(pr + 1) * 128],
                                srcs[k][0:cl, 0:N],
                                start=first, stop=False)
                            first = False
                    for par in range(2):
                        b = 2 * pr + par
                        nc.tensor.matmul(
                            pc[par * 64:(par + 1) * 64, 0:N],
                            ca0[0:128, 0:64],
                            xr_t[:, b * NB:b * NB + N],
                            start=False, stop=(par == 1))
                    for par in range(2):
                        b = 2 * pr + par
                        sl = slice(b * NB, (b + 1) * NB)
                        nc.scalar.activation(
                            c_t[0:64, b * NB:b * NB + N],
                            pc[par * 64:(par + 1) * 64, 0:N], AFT.Tanh,
                            bias=cb2[par * 64:(par + 1) * 64, 0:1])
                        # GRU per batch (A1, fp16 on DVE), pipelines w/ tanh:
                        # d = h - c -> xr[0:64] (rh dead); m = u*d; h' = c + m
                        nc.vector.tensor_tensor(xr_t[0:64, sl], xh_t[0:64, sl],
                                                c_t[0:64, sl], ALU.subtract)
                        nc.vector.tensor_tensor(xr_t[0:64, sl],
                                                uu[hf][0:64, sl],
                                                xr_t[0:64, sl], ALU.mult)
                        nc.vector.tensor_tensor(xh_t[0:64, sl], c_t[0:64, sl],
                                                xr_t[0:64, sl], ALU.add)

            def mirror(src_cell, dst_cell, hf):
                """Copy h' of src layer into x rows of dst layer tiles."""
                for b in range(HB):
                    sl = slice(b * NB, (b + 1) * NB)
                    nc.vector.tensor_copy(xh[(dst_cell, hf)][64:128, sl],
                                          xh[(src_cell, hf)][0:64, sl])
                    nc.vector.tensor_copy(xr[(dst_cell, hf)][64:128, sl],
                                          xh[(src_cell, hf)][0:64, sl])

            # ---- encoder ----
            xr3 = x_in[:].rearrange("t d (g f) -> t d g f", g=2)
            for t in range(nsteps_enc):
                for hf in range(2):
                    nc.sync.dma_start(out=xh[("enc0", hf)][64:66, :],
                                      in_=xr3[t, :, hf, :])
                    nc.sync.dma_start(out=xr[("enc0", hf)][64:66, :],
                                      in_=xr3[t, :, hf, :])
                for hf in range(2):
                    cell("enc0", hf)
                    mirror("enc0", "enc1", hf)
                    cell("enc1", hf)

            # ---- copy encoder state to decoder ----
            for hf in range(2):
                nc.vector.tensor_copy(xh[("dec0", hf)][0:64, :],
                                      xh[("enc0", hf)][0:64, :])
                nc.vector.tensor_copy(xh[("dec1", hf)][0:64, :],
                                      xh[("enc1", hf)][0:64, :])

            # ---- decoder ----
            for t in range(nsteps_dec):
                for hf in range(2):
                    cell("dec0", hf)
                    mirror("dec0", "dec1", hf)
                    cell("dec1", hf)
                    # projection: out = h_dec1 @ pW + pb
                    pf = projf[hf]
                    for b in range(HB):
                        pp = psp.tile([1, 384], F32, tag="stage")
                        nc.tensor.matmul(
                            pp[0:1, 0:384],
                            wt["pW"][0:64, 0:1],
                            xh[("dec1", hf)][0:64, b * NB:(b + 1) * NB],
                            start=True, stop=True)
                        nc.scalar.activation(
                            pf[0:1, b * NB:(b + 1) * NB],
                            pp[0:1, 0:384], AFT.Identity,
                            bias=wt["pb128"][0:1, 0:1])
                    ov = pf[0:1, :].rearrange("p (b n) -> p b n", b=HB)
                    nc.sync.dma_start(out=out_d[t][:, hf * HB:(hf + 1) * HB, :],
                                      in_=ov[:, :, 0:N])
                    if t < nsteps_dec - 1:
                        # feedback: proj -> x rows of dec0
                        for b in range(HB):
                            nc.vector.tensor_copy(
                                xh[("dec0", hf)][64:65, b * NB:(b + 1) * NB],
                                pf[0:1, b * NB:(b + 1) * NB])
                        nc.vector.tensor_copy(xr[("dec0", hf)][64:65, :],
                                              xh[("dec0", hf)][64:65, :])

    nc.finalize()
    _BUILD_CACHE[key] = nc
    return nc


def _prep_inputs(inputs, support, weights):
    """Host-side prep. Returns (shared_map, per_core_x list)."""
    s32 = np.asarray(support, np.float32)
    s2_32 = s32 @ s32
    shared = {}
    for nm, m in (("s", s32), ("s2", s2_32)):
        chunks = np.zeros((3, 128, N), np.float16)
        for ci, (c0, cl) in enumerate(NCH):
            chunks[ci, 0:cl, :] = m[c0:c0 + cl, :].astype(np.float16)
        shared[nm] = chunks
    for c in CELLS:
        din = CELL_DIN[c]
        ga0, gw1, gw2 = _pad_w(weights[f"{c}_gate_W"], din, 2 * U)
        ca0, cw1, cw2 = _pad_w(weights[f"{c}_cand_W"], din, U)
        gb = np.zeros((128, 1), np.float32)
        gb[:, 0] = weights[f"{c}_gate_b"]
        cb2 = np.zeros((128, 1), np.float32)
        cb2[0:64, 0] = weights[f"{c}_cand_b"]
        cb2[64:128, 0] = weights[f"{c}_cand_b"]
        shared.update({f"{c}_gA0": ga0, f"{c}_gW1": gw1, f"{c}_gW2": gw2,
                       f"{c}_cA0": ca0, f"{c}_cW1": cw1, f"{c}_cW2": cw2,
                       f"{c}_gb": gb, f"{c}_cb2": cb2})
    shared["pW"] = np.ascontiguousarray(weights["proj_W"]).astype(np.float16)
    pb128 = np.zeros((128, 1), np.float32)
    pb128[:, 0] = float(np.asarray(weights["proj_b"]).reshape(-1)[0])
    shared["pb128"] = pb128

    # inputs (T, B, N*DIN) -> per-core (T, DIN, AF) with node padding
    x = np.asarray(inputs, np.float32).reshape(T, B, N, DIN)
    per_core = []
    for c in range(NCORES):
        xc = x[:, c * BL:(c + 1) * BL]                  # (T, BL, N, DIN)
        xp = np.zeros((T, DIN, BL, NB), np.float16)
        xp[:, :, :, 0:N] = xc.transpose(0, 3, 1, 2)
        per_core.append(xp.reshape(T, DIN, AF))
    return shared, per_core


def kernel(**inputs) -> np.ndarray:
    support = np.asarray(inputs["support"], np.float32)
    weights = {k: np.asarray(v, np.float32) for k, v in inputs.items()
               if k not in ("inputs", "support")}
    shared, per_core_x = _prep_inputs(inputs["inputs"], support, weights)

    nc = _build(T, HZ)
    if os.environ.get("DCRNN_TRACE"):
        _install_ntff_hook()
    in_maps = [dict(shared, x=per_core_x[c]) for c in range(NCORES)]
    res = run_bass_kernel_spmd(nc, in_maps, list(range(NCORES)),
                               trace=bool(os.environ.get("DCRNN_TRACE")))
    global LAST_RESULT
    LAST_RESULT = res
    if res.exec_time_ns is not None:
        print(f"HW exec time: {res.exec_time_ns} ns")
    outs = [res.results[c]["out"].reshape(HZ, BL, N) for c in range(NCORES)]
    return np.concatenate(outs, axis=1).astype(np.float32)


if __name__ == "__main__":
    sys.path.insert(0, "/root/problem")
    import reference
    ins = reference.setup_inputs()
    ins = {k: np.asarray(v) for k, v in ins.items()}
    exp = np.asarray(reference.reference(**ins))
    act = kernel(**ins)
    err = np.max(np.abs(act - exp)) / (np.abs(exp).max() + 1e-30)
    print("Relative error:", err)


# revision 19
# speedup vs baseline: 2.3760x; 1.1486x over previous
"""DCRNN (PEMS-BAY) Trainium2 Bass kernel, data-parallel over batch on 8 cores.

Transpose-free gconv via S^2 precompute, fp16 matmuls/states, fp32 psum.

Layouts per core (local batch BL=8, split in 2 halves of HB=4):
  A1: [feature partitions, b*384 + n]   (state tiles XH/XR: rows 0:64 = h|rh,
      rows 64:64+din = x)
  B:  [node-chunk partitions (128/128/69), b*Fout + f]  (W-product tiles)
gconv:  pre = X@A0 + S@(X@W1) + S^2@(X@(2*W2))      [A0 = W0 - W2]
  P2B/P1B = direct-to-B W-matmuls (lhsT = XH col-slice, rhs = weight);
  preact accumulated per batch in one psum bank: P0 (lhsT=A0, rhs=XH)
  start=True, then S@P1B + S2@P2B with S/S2 as *rhs* (lhsT = P_B chunk)
  which lands back in A1 layout.  No PE transposes anywhere.
Cand gconv packs 2 batches into 128 partitions ((b%2)*64+f) per psum bank.
"""
import sys
import os
import numpy as np

sys.path.insert(0, "/opt/trn_rl_repo")

import concourse.bass as bass  # noqa: E402
import concourse.mybir as mybir  # noqa: E402
import concourse.tile as tile  # noqa: E402
from concourse import bacc  # noqa: E402
from concourse.bass_utils import run_bass_kernel_spmd  # noqa: E402

# problem constants
N = 325
B = 64
T = 12
HZ = 12
U = 64
DIN = 2
DOUT = 1
NCORES = 8
BL = B // NCORES          # 8 local batch
NB = 384                  # padded node stride per batch
AF = BL * NB              # 3072 A-layout free width
NCH = [(0, 128), (128, 128), (256, 69)]   # node chunks (offset, len)
HB = BL // 2              # 4 batches per half
HAF = HB * NB             # 1536 A cols per half

F32 = mybir.dt.float32
F16 = mybir.dt.float16
AFT = mybir.ActivationFunctionType
ALU = mybir.AluOpType

CELLS = ["enc0", "enc1", "dec0", "dec1"]
CELL_DIN = {"enc0": DIN, "enc1": U, "dec0": DOUT, "dec1": U}

_BUILD_CACHE = {}
LAST_RESULT = None


def _install_ntff_hook():
    """Register the axon NTFF profiling hook if the image lacks antenv.axon_hooks."""
    import types
    import antenv
    if getattr(antenv, "axon_hooks", None) is not None:
        return
    m = types.ModuleType("antenv.axon_hooks")
    state = {"h": None}
    m.set_axon_ntff_profile_hook = lambda h: state.__setitem__("h", h)
    m.get_axon_ntff_profile_hook = lambda: state["h"]
    sys.modules["antenv.axon_hooks"] = m
    antenv.axon_hooks = m
    try:
        from trn_agent_boot.trn_boot import _ntff_profile_via_ctypes
        hook = _ntff_profile_via_ctypes("/opt/axon/libaxon_pjrt.so")
        if hook is not None:
            m.set_axon_ntff_profile_hook(hook)
    except Exception:
        pass


def _pad_w(w, din, fout):
    """(3F, fout) -> three [128, fout] fp16 mats A0, W1, 2*W2.

    Padded row map: rows 0:64 <- h/rh features (orig rows din:F),
    rows 64:64+din <- x features (orig rows 0:din). Others zero.
    """
    f = din + U
    w0, w1, w2 = w[0:f], w[f:2 * f], w[2 * f:3 * f]

    def pad(m):
        p = np.zeros((128, fout), np.float32)
        p[0:64] = m[din:f]
        p[64:64 + din] = m[0:din]
        return p.astype(np.float16)

    return pad(w0 - w2), pad(w1), pad(2.0 * w2)


def _build(nsteps_enc, nsteps_dec):
    key = (nsteps_enc, nsteps_dec)
    if key in _BUILD_CACHE:
        return _BUILD_CACHE[key]

    nc = bacc.Bacc()
    # ---- DRAM params ----
    x_in = nc.declare_dram_parameter("x", [T, DIN, AF], F16, isOutput=False)
    s_in = nc.declare_dram_parameter("s", [3, 128, N], F16, isOutput=False)
    s2_in = nc.declare_dram_parameter("s2", [3, 128, N], F16, isOutput=False)
    wparams = {}
    for c in CELLS:
        for nm, shp, dt_ in [("gA0", [128, 128], F16), ("gW1", [128, 128], F16),
                             ("gW2", [128, 128], F16), ("cA0", [128, 64], F16),
                             ("cW1", [128, 64], F16), ("cW2", [128, 64], F16),
                             ("gb", [128, 1], F32), ("cb2", [128, 1], F32)]:
            wparams[f"{c}_{nm}"] = nc.declare_dram_parameter(
                f"{c}_{nm}", shp, dt_, isOutput=False)
    wparams["pW"] = nc.declare_dram_parameter("pW", [64, 1], F16, isOutput=False)
    wparams["pb128"] = nc.declare_dram_parameter("pb128", [128, 1], F32,
                                                 isOutput=False)
    out_d = nc.declare_dram_parameter("out", [HZ, 1, BL, N], F32, isOutput=True)

    with tile.TileContext(nc) as tc:
        with tc.tile_pool(name="const", bufs=1) as cp, \
             tc.tile_pool(name="state", bufs=1) as st, \
             tc.tile_pool(name="bprod", bufs=1) as bp, \
             tc.tile_pool(name="pstage", bufs=3, space="PSUM") as psp, \
             tc.tile_pool(name="pgate", bufs=3, space="PSUM") as pgp, \
             tc.tile_pool(name="pcand", bufs=2, space="PSUM") as pcp:

            # ---- constants to SBUF ----
            wt = {}
            for c in CELLS:
                for nm in ["gA0", "gW1", "gW2"]:
                    wt[f"{c}_{nm}"] = cp.tile([128, 128], F16, tag=f"{c}_{nm}",
                                              name=f"{c}_{nm}")
                for nm in ["cA0", "cW1", "cW2"]:
                    wt[f"{c}_{nm}"] = cp.tile([128, 64], F16, tag=f"{c}_{nm}",
                                              name=f"{c}_{nm}")
                wt[f"{c}_gb"] = cp.tile([128, 1], F32, tag=f"{c}_gb",
                                        name=f"{c}_gb")
                wt[f"{c}_cb2"] = cp.tile([128, 1], F32, tag=f"{c}_cb2",
                                         name=f"{c}_cb2")
            wt["pW"] = cp.tile([64, 1], F16, tag="pW", name="pW")
            wt["pb128"] = cp.tile([128, 1], F32, tag="pb128", name="pb128")
            for k, t in wt.items():
                nc.sync.dma_start(out=t, in_=wparams[k][:])
            s_t, s2_t = [], []
            for ci, (c0, cl) in enumerate(NCH):
                stl = cp.tile([128, N], F16, tag=f"s{ci}", name=f"s{ci}")
                nc.sync.dma_start(out=stl[0:cl, :], in_=s_in[ci, 0:cl, :])
                s_t.append(stl)
                s2l = cp.tile([128, N], F16, tag=f"s2{ci}", name=f"s2{ci}")
                nc.sync.dma_start(out=s2l[0:cl, :], in_=s2_in[ci, 0:cl, :])
                s2_t.append(s2l)

            # ---- state tiles ----
            xh, xr = {}, {}
            ru, ct, uu = {}, {}, {}
            for hf in range(2):
                for c in CELLS:
                    xh[(c, hf)] = st.tile([128, HAF], F16, tag=f"xh_{c}_{hf}",
                                          name=f"xh_{c}_{hf}")
                    xr[(c, hf)] = st.tile([128, HAF], F16, tag=f"xr_{c}_{hf}",
                                          name=f"xr_{c}_{hf}")
                for lv in range(2):
                    ru[(hf, lv)] = st.tile([128, HAF], F16, tag=f"ru{hf}{lv}",
                                           name=f"ru{hf}{lv}")
                    ct[(hf, lv)] = st.tile([64, HAF], F16, tag=f"ct{hf}{lv}",
                                           name=f"ct{hf}{lv}")
                    uu[(hf, lv)] = st.tile([64, HAF], F16, tag=f"uu{hf}{lv}",
                                           name=f"uu{hf}{lv}")

            projf = {hf: st.tile([1, HAF], F32, tag=f"projf{hf}",
                                 name=f"projf{hf}") for hf in range(2)}

            for tl in (list(xh.values()) + list(xr.values())
                       + list(ru.values()) + list(ct.values())
                       + list(uu.values())):
                nc.vector.memset(tl[:, :], 0.0)
            tc.strict_bb_all_engine_barrier()

            def wprod(cell, hf, src_t, wprefix, fout):
                """Direct-to-B W-matmuls: out[(n-chunk), b*fout+f] tiles.

                Returns 3 SBUF fp16 tiles [cl, HB*fout]. evac engines:
                W2-product on Act, W1-product on Pool.
                """
                w2 = wt[f"{cell}_{wprefix}W2"]
                w1 = wt[f"{cell}_{wprefix}W1"]
                fw = HB * fout
                out = {}
                for role, w_ in (("p2", w2), ("p1", w1)):
                    tiles = []
                    for ci, (c0, cl) in enumerate(NCH):
                        stg = psp.tile([128, 512], F32, tag="stage")
                        for b in range(HB):
                            nc.tensor.matmul(
                                stg[0:cl, b * fout:(b + 1) * fout],
                                src_t[:, b * NB + c0: b * NB + c0 + cl],
                                w_[0:128, 0:fout],
                                start=(b == 0), stop=(b == HB - 1))
                        dst = bp.tile([128, fw], F16,
                                      tag=f"{cell}{wprefix}{role}{ci}_{hf}",
                                      name=f"{cell}{wprefix}{role}{ci}_{hf}")
                        if role == "p2":
                            nc.scalar.copy(dst[0:cl, 0:fw], stg[0:cl, 0:fw])
                        else:
                            nc.vector.tensor_copy(dst[0:cl, 0:fw],
                                                  stg[0:cl, 0:fw])
                        tiles.append(dst)
                    out[role] = tiles
                return out["p2"], out["p1"]

            def cell_phases(cname, hf, mirror_to=None):
                """One DCGRU cell on one batch-half, as 4 phase thunks.

                mirror_to: layer name whose x rows receive h' per batch.
                """
                xh_t = xh[(cname, hf)]
                xr_t = xr[(cname, hf)]
                gb = wt[f"{cname}_gb"]
                cb2 = wt[f"{cname}_cb2"]
                lv = 0 if cname in ("enc0", "dec0") else 1
                ru_t, c_t, uu_t = ru[(hf, lv)], ct[(hf, lv)], uu[(hf, lv)]
                box = {}

                def ph1():     # gate W-products
                    box["g"] = wprod(cname, hf, xh_t, "g", 128)

                def ph2():     # gate psums + sigmoid + rh + uu
                    p2b, p1b = box["g"]
                    a0 = wt[f"{cname}_gA0"]
                    for b in range(HB):
                        pg = pgp.tile([128, 384], F32, tag="pgb")
                        nc.tensor.matmul(pg[0:128, 0:N], a0[0:128, 0:128],
                                         xh_t[:, b * NB:b * NB + N],
                                         start=True, stop=False)
                        for pt, srcs in ((p1b, s_t), (p2b, s2_t)):
                            for k, (c0, cl) in enumerate(NCH):
                                last = pt is p2b and k == 2
                                nc.tensor.matmul(
                                    pg[0:128, 0:N],
                                    pt[k][0:cl, b * 128:(b + 1) * 128],
                                    srcs[k][0:cl, 0:N],
                                    start=False, stop=last)
                        nc.scalar.activation(ru_t[:, b * NB:b * NB + N],
                                             pg[0:128, 0:N], AFT.Sigmoid,
                                             bias=gb[0:128, 0:1])
                        sl = slice(b * NB, (b + 1) * NB)
                        nc.vector.tensor_tensor(xr_t[0:64, sl],
                                                ru_t[0:64, sl],
                                                xh_t[0:64, sl], ALU.mult)
                        nc.vector.tensor_copy(uu_t[0:64, sl],
                                              ru_t[64:128, sl])

                def ph3():     # cand W-products
                    box["c"] = wprod(cname, hf, xr_t, "c", 64)

                def ph4():     # cand psums + tanh + GRU (+ mirror)
                    q2b, q1b = box["c"]
                    ca0 = wt[f"{cname}_cA0"]
                    for pr in range(2):
                        pc = pcp.tile([128, 384], F32, tag="pcb")
                        first = True
                        for pt, srcs in ((q1b, s_t), (q2b, s2_t)):
                            for k, (c0, cl) in enumerate(NCH):
                                nc.tensor.matmul(
                                    pc[0:128, 0:N],
                                    pt[k][0:cl, pr * 128:(pr + 1) * 128],
                                    srcs[k][0:cl, 0:N],
                                    start=first, stop=False)
                                first = False
                        for par in range(2):
                            b = 2 * pr + par
                            nc.tensor.matmul(
                                pc[par * 64:(par + 1) * 64, 0:N],
                                ca0[0:128, 0:64],
                                xr_t[:, b * NB:b * NB + N],
                                start=False, stop=(par == 1))
                        for par in range(2):
                            b = 2 * pr + par
                            sl = slice(b * NB, (b + 1) * NB)
                            nc.scalar.activation(
                                c_t[0:64, b * NB:b * NB + N],
                                pc[par * 64:(par + 1) * 64, 0:N], AFT.Tanh,
                                bias=cb2[par * 64:(par + 1) * 64, 0:1])
                            # GRU: d = h - c -> xr[0:64]; m = u*d; h' = c + m
                            nc.vector.tensor_tensor(xr_t[0:64, sl],
                                                    xh_t[0:64, sl],
                                                    c_t[0:64, sl],
                                                    ALU.subtract)
                            nc.vector.tensor_tensor(xr_t[0:64, sl],
                                                    uu_t[0:64, sl],
                                                    xr_t[0:64, sl], ALU.mult)
                            nc.vector.tensor_tensor(xh_t[0:64, sl],
                                                    c_t[0:64, sl],
                                                    xr_t[0:64, sl], ALU.add)
                            if mirror_to is not None:
                                nc.vector.tensor_copy(
                                    xh[(mirror_to, hf)][64:128, sl],
                                    xh_t[0:64, sl])
                                nc.vector.tensor_copy(
                                    xr[(mirror_to, hf)][64:128, sl],
                                    xh_t[0:64, sl])

                return [ph1, ph2, ph3, ph4]

            def interleave(*phase_lists):
                """Emit phase thunks round-robin: software-pipelines the
                independent cell streams so the PE queue never head-of-line
                blocks on one stream's evac/activation latency."""
                if os.environ.get("DCRNN_NO_PIPELINE"):
                    for pl in phase_lists:
                        for p in pl:
                            p()
                    return
                for i in range(max(len(p) for p in phase_lists)):
                    for pl in phase_lists:
                        if i < len(pl):
                            pl[i]()

            def proj_phase(hf, t):
                """Projection + output DMA + decoder feedback thunk.

                Must be emitted AFTER dec1's ph4 (GRU) — emission order
                defines the dataflow, so an early emit would read stale h.
                """
                def ph():
                    pf = projf[hf]
                    for b in range(HB):
                        pp = psp.tile([1, 384], F32, tag="stage")
                        nc.tensor.matmul(
                            pp[0:1, 0:384],
                            wt["pW"][0:64, 0:1],
                            xh[("dec1", hf)][0:64, b * NB:(b + 1) * NB],
                            start=True, stop=True)
                        nc.scalar.activation(
                            pf[0:1, b * NB:(b + 1) * NB],
                            pp[0:1, 0:384], AFT.Identity,
                            bias=wt["pb128"][0:1, 0:1])
                        if t < nsteps_dec - 1:
                            # feedback: proj -> x rows of dec0
                            nc.vector.tensor_copy(
                                xh[("dec0", hf)][64:65, b * NB:(b + 1) * NB],
                                pf[0:1, b * NB:(b + 1) * NB])
                            nc.vector.tensor_copy(
                                xr[("dec0", hf)][64:65, b * NB:(b + 1) * NB],
                                pf[0:1, b * NB:(b + 1) * NB])
                    ov = pf[0:1, :].rearrange("p (b n) -> p b n", b=HB)
                    nc.sync.dma_start(out=out_d[t][:, hf * HB:(hf + 1) * HB, :],
                                      in_=ov[:, :, 0:N])
                return [ph]

            # ---- encoder: enc1(t-1) pipelined against enc0(t) ----
            xr3 = x_in[:].rearrange("t d (g f) -> t d g f", g=2)
            prev_l1 = []
            for t in range(nsteps_enc):
                for hf in range(2):
                    nc.sync.dma_start(out=xh[("enc0", hf)][64:66, :],
                                      in_=xr3[t, :, hf, :])
                    nc.sync.dma_start(out=xr[("enc0", hf)][64:66, :],
                                      in_=xr3[t, :, hf, :])
                cur_l0 = [cell_phases("enc0", hf, mirror_to="enc1")
                          for hf in range(2)]
                interleave(*(prev_l1 + cur_l0))
                prev_l1 = [cell_phases("enc1", hf) for hf in range(2)]

            # ---- last enc1 || copy encoder state to decoder ----
            interleave(*prev_l1)
            for hf in range(2):
                nc.vector.tensor_copy(xh[("dec0", hf)][0:64, :],
                                      xh[("enc0", hf)][0:64, :])
                nc.vector.tensor_copy(xh[("dec1", hf)][0:64, :],
                                      xh[("enc1", hf)][0:64, :])

            # ---- decoder (serial: dec0 -> dec1 -> proj feedback) ----
            for t in range(nsteps_dec):
                interleave(*[cell_phases("dec0", hf, mirror_to="dec1")
                             for hf in range(2)])
                interleave(*([cell_phases("dec1", hf) for hf in range(2)]
                             + [proj_phases(hf, t) for hf in range(2)]))

    nc.finalize()
    _BUILD_CACHE[key] = nc
    return nc


def _prep_inputs(inputs, support, weights):
    """Host-side prep. Returns (shared_map, per_core_x list)."""
    s32 = np.asarray(support, np.float32)
    s2_32 = s32 @ s32
    shared = {}
    for nm, m in (("s", s32), ("s2", s2_32)):
        chunks = np.zeros((3, 128, N), np.float16)
        for ci, (c0, cl) in enumerate(NCH):
            chunks[ci, 0:cl, :] = m[c0:c0 + cl, :].astype(np.float16)
        shared[nm] = chunks
    for c in CELLS:
        din = CELL_DIN[c]
        ga0, gw1, gw2 = _pad_w(weights[f"{c}_gate_W"], din, 2 * U)
        ca0, cw1, cw2 = _pad_w(weights[f"{c}_cand_W"], din, U)
        gb = np.zeros((128, 1), np.float32)
        gb[:, 0] = weights[f"{c}_gate_b"]
        cb2 = np.zeros((128, 1), np.float32)
        cb2[0:64, 0] = weights[f"{c}_cand_b"]
        cb2[64:128, 0] = weights[f"{c}_cand_b"]
        shared.update({f"{c}_gA0": ga0, f"{c}_gW1": gw1, f"{c}_gW2": gw2,
                       f"{c}_cA0": ca0, f"{c}_cW1": cw1, f"{c}_cW2": cw2,
                       f"{c}_gb": gb, f"{c}_cb2": cb2})
    shared["pW"] = np.ascontiguousarray(weights["proj_W"]).astype(np.float16)
    pb128 = np.zeros((128, 1), np.float32)
    pb128[:, 0] = float(np.asarray(weights["proj_b"]).reshape(-1)[0])
    shared["pb128"] = pb128

    # inputs (T, B, N*DIN) -> per-core (T, DIN, AF) with node padding
    x = np.asarray(inputs, np.float32).reshape(T, B, N, DIN)
    per_core = []
    for c in range(NCORES):
        xc = x[:, c * BL:(c + 1) * BL]                  # (T, BL, N, DIN)
        xp = np.zeros((T, DIN, BL, NB), np.float16)
        xp[:, :, :, 0:N] = xc.transpose(0, 3, 1, 2)
        per_core.append(xp.reshape(T, DIN, AF))
    return shared, per_core


def kernel(**inputs) -> np.ndarray:
    support = np.asarray(inputs["support"], np.float32)
    weights = {k: np.asarray(v, np.float32) for k, v in inputs.items()
               if k not in ("inputs", "support")}
    shared, per_core_x = _prep_inputs(inputs["inputs"], support, weights)

    nc = _build(T, HZ)
    if os.environ.get("DCRNN_TRACE"):
        _install_ntff_hook()
    in_maps = [dict(shared, x=per_core_x[c]) for c in range(NCORES)]
    res = run_bass_kernel_spmd(nc, in_maps, list(range(NCORES)),
                               trace=bool(os.environ.get("DCRNN_TRACE")))
    global LAST_RESULT
    LAST_RESULT = res
    if res.exec_time_ns is not None:
        print(f"HW exec time: {res.exec_time_ns} ns")
    outs = [res.results[c]["out"].reshape(HZ, BL, N) for c in range(NCORES)]
    return np.concatenate(outs, axis=1).astype(np.float32)


if __name__ == "__main__":
    sys.path.insert(0, "/root/problem")
    import reference
    ins = reference.setup_inputs()
    ins = {k: np.asarray(v) for k, v in ins.items()}
    exp = np.asarray(reference.reference(**ins))
    act = kernel(**ins)
    err = np.max(np.abs(act - exp)) / (np.abs(exp).max() + 1e-30)
    print("Relative error:", err)


# revision 21
# speedup vs baseline: 2.9707x; 1.2503x over previous
"""DCRNN (PEMS-BAY) Trainium2 Bass kernel, data-parallel over batch on 8 cores.

Transpose-free gconv via S^2 precompute, fp16 matmuls/states, fp32 psum.

Layouts per core (local batch BL=8, split in 2 halves of HB=4):
  A1: [feature partitions, b*384 + n]   (state tiles XH/XR: rows 0:64 = h|rh,
      rows 64:64+din = x)
  B:  [node-chunk partitions (128/128/69), b*Fout + f]  (W-product tiles)
gconv:  pre = X@A0 + S@(X@W1) + S^2@(X@(2*W2))      [A0 = W0 - W2]
  P2B/P1B = direct-to-B W-matmuls (lhsT = XH col-slice, rhs = weight);
  preact accumulated per batch in one psum bank: P0 (lhsT=A0, rhs=XH)
  start=True, then S@P1B + S2@P2B with S/S2 as *rhs* (lhsT = P_B chunk)
  which lands back in A1 layout.  No PE transposes anywhere.
Cand gconv packs 2 batches into 128 partitions ((b%2)*64+f) per psum bank.
"""
import sys
import os
import numpy as np

sys.path.insert(0, "/opt/trn_rl_repo")

import concourse.bass as bass  # noqa: E402
import concourse.mybir as mybir  # noqa: E402
import concourse.tile as tile  # noqa: E402
from concourse import bacc  # noqa: E402
from concourse.bass_utils import run_bass_kernel_spmd  # noqa: E402

# problem constants
N = 325
B = 64
T = 12
HZ = 12
U = 64
DIN = 2
DOUT = 1
NCORES = 8
BL = B // NCORES          # 8 local batch
NB = 384                  # padded node stride per batch
AF = BL * NB              # 3072 A-layout free width
NCH = [(0, 128), (128, 128), (256, 69)]   # node chunks (offset, len)
HB = BL // 2              # 4 batches per half
HAF = HB * NB             # 1536 A cols per half

F32 = mybir.dt.float32
F16 = mybir.dt.float16
AFT = mybir.ActivationFunctionType
ALU = mybir.AluOpType

CELLS = ["enc0", "enc1", "dec0", "dec1"]
CELL_DIN = {"enc0": DIN, "enc1": U, "dec0": DOUT, "dec1": U}

_BUILD_CACHE = {}
LAST_RESULT = None


def _install_ntff_hook():
    """Register the axon NTFF profiling hook if the image lacks antenv.axon_hooks."""
    import types
    import antenv
    if getattr(antenv, "axon_hooks", None) is not None:
        return
    m = types.ModuleType("antenv.axon_hooks")
    state = {"h": None}
    m.set_axon_ntff_profile_hook = lambda h: state.__setitem__("h", h)
    m.get_axon_ntff_profile_hook = lambda: state["h"]
    sys.modules["antenv.axon_hooks"] = m
    antenv.axon_hooks = m
    try:
        from trn_agent_boot.trn_boot import _ntff_profile_via_ctypes
        hook = _ntff_profile_via_ctypes("/opt/axon/libaxon_pjrt.so")
        if hook is not None:
            m.set_axon_ntff_profile_hook(hook)
    except Exception:
        pass


def _pad_w(w, din, fout):
    """(3F, fout) -> three [128, fout] fp16 mats A0, W1, 2*W2.

    Padded row map: rows 0:64 <- h/rh features (orig rows din:F),
    rows 64:64+din <- x features (orig rows 0:din). Others zero.
    """
    f = din + U
    w0, w1, w2 = w[0:f], w[f:2 * f], w[2 * f:3 * f]

    def pad(m):
        p = np.zeros((128, fout), np.float32)
        p[0:64] = m[din:f]
        p[64:64 + din] = m[0:din]
        return p.astype(np.float16)

    return pad(w0 - w2), pad(w1), pad(2.0 * w2)


def _build(nsteps_enc, nsteps_dec):
    key = (nsteps_enc, nsteps_dec)
    if key in _BUILD_CACHE:
        return _BUILD_CACHE[key]

    nc = bacc.Bacc()
    # ---- DRAM params ----
    x_in = nc.declare_dram_parameter("x", [T, DIN, AF], F16, isOutput=False)
    s_in = nc.declare_dram_parameter("s", [3, 128, N], F16, isOutput=False)
    s2_in = nc.declare_dram_parameter("s2", [3, 128, N], F16, isOutput=False)
    wparams = {}
    for c in CELLS:
        for nm, shp, dt_ in [("gA0", [128, 128], F16), ("gW1", [128, 128], F16),
                             ("gW2", [128, 128], F16), ("cA0", [128, 64], F16),
                             ("cW1", [128, 64], F16), ("cW2", [128, 64], F16),
                             ("gb", [128, 1], F32), ("cb2", [128, 1], F32)]:
            wparams[f"{c}_{nm}"] = nc.declare_dram_parameter(
                f"{c}_{nm}", shp, dt_, isOutput=False)
    wparams["pW"] = nc.declare_dram_parameter("pW", [64, 1], F16, isOutput=False)
    wparams["pb128"] = nc.declare_dram_parameter("pb128", [128, 1], F32,
                                                 isOutput=False)
    out_d = nc.declare_dram_parameter("out", [HZ, 1, BL, N], F32, isOutput=True)

    with tile.TileContext(nc) as tc:
        with tc.tile_pool(name="const", bufs=1) as cp, \
             tc.tile_pool(name="state", bufs=1) as st, \
             tc.tile_pool(name="bprod", bufs=1) as bp, \
             tc.tile_pool(name="pstage", bufs=3, space="PSUM") as psp, \
             tc.tile_pool(name="pgate", bufs=3, space="PSUM") as pgp, \
             tc.tile_pool(name="pcand", bufs=2, space="PSUM") as pcp:

            # ---- constants to SBUF ----
            wt = {}
            for c in CELLS:
                for nm in ["gA0", "gW1", "gW2"]:
                    wt[f"{c}_{nm}"] = cp.tile([128, 128], F16, tag=f"{c}_{nm}",
                                              name=f"{c}_{nm}")
                for nm in ["cA0", "cW1", "cW2"]:
                    wt[f"{c}_{nm}"] = cp.tile([128, 64], F16, tag=f"{c}_{nm}",
                                              name=f"{c}_{nm}")
                wt[f"{c}_gb"] = cp.tile([128, 1], F32, tag=f"{c}_gb",
                                        name=f"{c}_gb")
                wt[f"{c}_cb2"] = cp.tile([128, 1], F32, tag=f"{c}_cb2",
                                         name=f"{c}_cb2")
            wt["pW"] = cp.tile([64, 1], F16, tag="pW", name="pW")
            wt["pb128"] = cp.tile([128, 1], F32, tag="pb128", name="pb128")
            for k, t in wt.items():
                nc.sync.dma_start(out=t, in_=wparams[k][:])
            s_t, s2_t = [], []
            for ci, (c0, cl) in enumerate(NCH):
                stl = cp.tile([128, N], F16, tag=f"s{ci}", name=f"s{ci}")
                nc.sync.dma_start(out=stl[0:cl, :], in_=s_in[ci, 0:cl, :])
                s_t.append(stl)
                s2l = cp.tile([128, N], F16, tag=f"s2{ci}", name=f"s2{ci}")
                nc.sync.dma_start(out=s2l[0:cl, :], in_=s2_in[ci, 0:cl, :])
                s2_t.append(s2l)

            # ---- state tiles ----
            xh, xr = {}, {}
            ru, ct, uu = {}, {}, {}
            for hf in range(2):
                for c in CELLS:
                    xh[(c, hf)] = st.tile([128, HAF], F16, tag=f"xh_{c}_{hf}",
                                          name=f"xh_{c}_{hf}")
                    xr[(c, hf)] = st.tile([128, HAF], F16, tag=f"xr_{c}_{hf}",
                                          name=f"xr_{c}_{hf}")
                for lv in range(2):
                    ru[(hf, lv)] = st.tile([128, HAF], F16, tag=f"ru{hf}{lv}",
                                           name=f"ru{hf}{lv}")
                    ct[(hf, lv)] = st.tile([64, HAF], F16, tag=f"ct{hf}{lv}",
                                           name=f"ct{hf}{lv}")
                    uu[(hf, lv)] = st.tile([64, HAF], F16, tag=f"uu{hf}{lv}",
                                           name=f"uu{hf}{lv}")

            projf = {hf: st.tile([1, HAF], F32, tag=f"projf{hf}",
                                 name=f"projf{hf}") for hf in range(2)}

            for tl in (list(xh.values()) + list(xr.values())
                       + list(ru.values()) + list(ct.values())
                       + list(uu.values())):
                nc.vector.memset(tl[:, :], 0.0)
            tc.strict_bb_all_engine_barrier()

            def wprod(cell, hf, src_t, wprefix, fout):
                """Direct-to-B W-matmuls: out[(n-chunk), b*fout+f] tiles.

                Returns 3 SBUF fp16 tiles [cl, HB*fout]. evac engines:
                W2-product on Act, W1-product on Pool.
                """
                w2 = wt[f"{cell}_{wprefix}W2"]
                w1 = wt[f"{cell}_{wprefix}W1"]
                fw = HB * fout
                out = {}
                for role, w_ in (("p2", w2), ("p1", w1)):
                    tiles = []
                    for ci, (c0, cl) in enumerate(NCH):
                        stg = psp.tile([128, 512], F32, tag="stage")
                        for b in range(HB):
                            nc.tensor.matmul(
                                stg[0:cl, b * fout:(b + 1) * fout],
                                src_t[:, b * NB + c0: b * NB + c0 + cl],
                                w_[0:128, 0:fout],
                                start=(b == 0), stop=(b == HB - 1))
                        dst = bp.tile([128, fw], F16,
                                      tag=f"{cell}{wprefix}{role}{ci}_{hf}",
                                      name=f"{cell}{wprefix}{role}{ci}_{hf}")
                        if role == "p2":
                            nc.scalar.copy(dst[0:cl, 0:fw], stg[0:cl, 0:fw])
                        else:
                            nc.vector.tensor_copy(dst[0:cl, 0:fw],
                                                  stg[0:cl, 0:fw])
                        tiles.append(dst)
                    out[role] = tiles
                return out["p2"], out["p1"]

            def cell_phases(cname, hf, mirror_to=None):
                """One DCGRU cell on one batch-half, as 4 phase thunks.

                mirror_to: layer name whose x rows receive h' per batch.
                """
                xh_t = xh[(cname, hf)]
                xr_t = xr[(cname, hf)]
                gb = wt[f"{cname}_gb"]
                cb2 = wt[f"{cname}_cb2"]
                lv = 0 if cname in ("enc0", "dec0") else 1
                ru_t, c_t, uu_t = ru[(hf, lv)], ct[(hf, lv)], uu[(hf, lv)]
                box = {}

                def ph1():     # gate W-products
                    box["g"] = wprod(cname, hf, xh_t, "g", 128)

                def ph2():     # gate psums + sigmoid + rh + uu
                    p2b, p1b = box["g"]
                    a0 = wt[f"{cname}_gA0"]
                    for b in range(HB):
                        pg = pgp.tile([128, 384], F32, tag="pgb")
                        nc.tensor.matmul(pg[0:128, 0:N], a0[0:128, 0:128],
                                         xh_t[:, b * NB:b * NB + N],
                                         start=True, stop=False)
                        for pt, srcs in ((p1b, s_t), (p2b, s2_t)):
                            for k, (c0, cl) in enumerate(NCH):
                                last = pt is p2b and k == 2
                                nc.tensor.matmul(
                                    pg[0:128, 0:N],
                                    pt[k][0:cl, b * 128:(b + 1) * 128],
                                    srcs[k][0:cl, 0:N],
                                    start=False, stop=last)
                        nc.scalar.activation(ru_t[:, b * NB:b * NB + N],
                                             pg[0:128, 0:N], AFT.Sigmoid,
                                             bias=gb[0:128, 0:1])
                        sl = slice(b * NB, (b + 1) * NB)
                        nc.vector.tensor_tensor(xr_t[0:64, sl],
                                                ru_t[0:64, sl],
                                                xh_t[0:64, sl], ALU.mult)
                        nc.vector.tensor_copy(uu_t[0:64, sl],
                                              ru_t[64:128, sl])

                def ph3():     # cand W-products
                    box["c"] = wprod(cname, hf, xr_t, "c", 64)

                def ph4():     # cand psums + tanh + GRU (+ mirror)
                    q2b, q1b = box["c"]
                    ca0 = wt[f"{cname}_cA0"]
                    for pr in range(2):
                        pc = pcp.tile([128, 384], F32, tag="pcb")
                        first = True
                        for pt, srcs in ((q1b, s_t), (q2b, s2_t)):
                            for k, (c0, cl) in enumerate(NCH):
                                nc.tensor.matmul(
                                    pc[0:128, 0:N],
                                    pt[k][0:cl, pr * 128:(pr + 1) * 128],
                                    srcs[k][0:cl, 0:N],
                                    start=first, stop=False)
                                first = False
                        for par in range(2):
                            b = 2 * pr + par
                            nc.tensor.matmul(
                                pc[par * 64:(par + 1) * 64, 0:N],
                                ca0[0:128, 0:64],
                                xr_t[:, b * NB:b * NB + N],
                                start=False, stop=(par == 1))
                        for par in range(2):
                            b = 2 * pr + par
                            sl = slice(b * NB, (b + 1) * NB)
                            nc.scalar.activation(
                                c_t[0:64, b * NB:b * NB + N],
                                pc[par * 64:(par + 1) * 64, 0:N], AFT.Tanh,
                                bias=cb2[par * 64:(par + 1) * 64, 0:1])
                            # GRU: d = h - c -> xr[0:64]; m = u*d; h' = c + m
                            nc.vector.tensor_tensor(xr_t[0:64, sl],
                                                    xh_t[0:64, sl],
                                                    c_t[0:64, sl],
                                                    ALU.subtract)
                            nc.vector.tensor_tensor(xr_t[0:64, sl],
                                                    uu_t[0:64, sl],
                                                    xr_t[0:64, sl], ALU.mult)
                            nc.vector.tensor_tensor(xh_t[0:64, sl],
                                                    c_t[0:64, sl],
                                                    xr_t[0:64, sl], ALU.add)
                            if mirror_to is not None:
                                nc.vector.tensor_copy(
                                    xh[(mirror_to, hf)][64:128, sl],
                                    xh_t[0:64, sl])
                                nc.vector.tensor_copy(
                                    xr[(mirror_to, hf)][64:128, sl],
                                    xh_t[0:64, sl])

                return [ph1, ph2, ph3, ph4]

            def interleave(*phase_lists):
                """Emit phase thunks round-robin: software-pipelines the
                independent cell streams so the PE queue never head-of-line
                blocks on one stream's evac/activation latency."""
                if os.environ.get("DCRNN_NO_PIPELINE"):
                    for pl in phase_lists:
                        for p in pl:
                            p()
                    return
                for i in range(max(len(p) for p in phase_lists)):
                    for pl in phase_lists:
                        if i < len(pl):
                            pl[i]()

            def proj_phase(hf, t):
                """Projection + output DMA + decoder feedback thunk.

                Must be emitted AFTER dec1's ph4 (GRU) — emission order
                defines the dataflow, so an early emit would read stale h.
                """
                def ph():
                    pf = projf[hf]
                    for b in range(HB):
                        pp = psp.tile([1, 384], F32, tag="stage")
                        nc.tensor.matmul(
                            pp[0:1, 0:384],
                            wt["pW"][0:64, 0:1],
                            xh[("dec1", hf)][0:64, b * NB:(b + 1) * NB],
                            start=True, stop=True)
                        nc.scalar.activation(
                            pf[0:1, b * NB:(b + 1) * NB],
                            pp[0:1, 0:384], AFT.Identity,
                            bias=wt["pb128"][0:1, 0:1])
                        if t < nsteps_dec - 1:
                            # feedback: proj -> x rows of dec0
                            nc.vector.tensor_copy(
                                xh[("dec0", hf)][64:65, b * NB:(b + 1) * NB],
                                pf[0:1, b * NB:(b + 1) * NB])
                            nc.vector.tensor_copy(
                                xr[("dec0", hf)][64:65, b * NB:(b + 1) * NB],
                                pf[0:1, b * NB:(b + 1) * NB])
                    ov = pf[0:1, :].rearrange("p (b n) -> p b n", b=HB)
                    nc.sync.dma_start(out=out_d[t][:, hf * HB:(hf + 1) * HB, :],
                                      in_=ov[:, :, 0:N])
                return ph

            # ---- encoder: enc1(t-1) pipelined against enc0(t) ----
            xr3 = x_in[:].rearrange("t d (g f) -> t d g f", g=2)
            prev_l1 = []
            for t in range(nsteps_enc):
                for hf in range(2):
                    nc.sync.dma_start(out=xh[("enc0", hf)][64:66, :],
                                      in_=xr3[t, :, hf, :])
                    nc.sync.dma_start(out=xr[("enc0", hf)][64:66, :],
                                      in_=xr3[t, :, hf, :])
                cur_l0 = [cell_phases("enc0", hf, mirror_to="enc1")
                          for hf in range(2)]
                interleave(*(prev_l1 + cur_l0))
                prev_l1 = [cell_phases("enc1", hf) for hf in range(2)]

            # ---- last enc1 || copy encoder state to decoder ----
            interleave(*prev_l1)
            for hf in range(2):
                nc.vector.tensor_copy(xh[("dec0", hf)][0:64, :],
                                      xh[("enc0", hf)][0:64, :])
                nc.vector.tensor_copy(xh[("dec1", hf)][0:64, :],
                                      xh[("enc1", hf)][0:64, :])

            # ---- decoder (serial: dec0 -> dec1 -> proj feedback) ----
            for t in range(nsteps_dec):
                interleave(*[cell_phases("dec0", hf, mirror_to="dec1")
                             for hf in range(2)])
                d1 = [cell_phases("dec1", hf) for hf in range(2)]
                for hf in range(2):
                    d1[hf].append(proj_phase(hf, t))
                interleave(*d1)

    nc.finalize()
    _BUILD_CACHE[key] = nc
    return nc


def _prep_inputs(inputs, support, weights):
    """Host-side prep. Returns (shared_map, per_core_x list)."""
    s32 = np.asarray(support, np.float32)
    s2_32 = s32 @ s32
    shared = {}
    for nm, m in (("s", s32), ("s2", s2_32)):
        chunks = np.zeros((3, 128, N), np.float16)
        for ci, (c0, cl) in enumerate(NCH):
            chunks[ci, 0:cl, :] = m[c0:c0 + cl, :].astype(np.float16)
        shared[nm] = chunks
    for c in CELLS:
        din = CELL_DIN[c]
        ga0, gw1, gw2 = _pad_w(weights[f"{c}_gate_W"], din, 2 * U)
        ca0, cw1, cw2 = _pad_w(weights[f"{c}_cand_W"], din, U)
        gb = np.zeros((128, 1), np.float32)
        gb[:, 0] = weights[f"{c}_gate_b"]
        cb2 = np.zeros((128, 1), np.float32)
        cb2[0:64, 0] = weights[f"{c}_cand_b"]
        cb2[64:128, 0] = weights[f"{c}_cand_b"]
        shared.update({f"{c}_gA0": ga0, f"{c}_gW1": gw1, f"{c}_gW2": gw2,
                       f"{c}_cA0": ca0, f"{c}_cW1": cw1, f"{c}_cW2": cw2,
                       f"{c}_gb": gb, f"{c}_cb2": cb2})
    shared["pW"] = np.ascontiguousarray(weights["proj_W"]).astype(np.float16)
    pb128 = np.zeros((128, 1), np.float32)
    pb128[:, 0] = float(np.asarray(weights["proj_b"]).reshape(-1)[0])
    shared["pb128"] = pb128

    # inputs (T, B, N*DIN) -> per-core (T, DIN, AF) with node padding
    x = np.asarray(inputs, np.float32).reshape(T, B, N, DIN)
    per_core = []
    for c in range(NCORES):
        xc = x[:, c * BL:(c + 1) * BL]                  # (T, BL, N, DIN)
        xp = np.zeros((T, DIN, BL, NB), np.float16)
        xp[:, :, :, 0:N] = xc.transpose(0, 3, 1, 2)
        per_core.append(xp.reshape(T, DIN, AF))
    return shared, per_core


def kernel(**inputs) -> np.ndarray:
    support = np.asarray(inputs["support"], np.float32)
    weights = {k: np.asarray(v, np.float32) for k, v in inputs.items()
               if k not in ("inputs", "support")}
    shared, per_core_x = _prep_inputs(inputs["inputs"], support, weights)

    nc = _build(T, HZ)
    if os.environ.get("DCRNN_TRACE"):
        _install_ntff_hook()
    in_maps = [dict(shared, x=per_core_x[c]) for c in range(NCORES)]
    res = run_bass_kernel_spmd(nc, in_maps, list(range(NCORES)),
                               trace=bool(os.environ.get("DCRNN_TRACE")))
    global LAST_RESULT
    LAST_RESULT = res
    if res.exec_time_ns is not None:
        print(f"HW exec time: {res.exec_time_ns} ns")
    outs = [res.results[c]["out"].reshape(HZ, BL, N) for c in range(NCORES)]
    return np.concatenate(outs, axis=1).astype(np.float32)


if __name__ == "__main__":
    sys.path.insert(0, "/root/problem")
    import reference
    ins = reference.setup_inputs()
    ins = {k: np.asarray(v) for k, v in ins.items()}
    exp = np.asarray(reference.reference(**ins))
    act = kernel(**ins)
    err = np.max(np.abs(act - exp)) / (np.abs(exp).max() + 1e-30)
    print("Relative error:", err)
